# revision 15
# baseline (speedup 1.0000x reference)
"""GAT (2-layer, PyG-style) Trainium2 Bass kernel, 8-core SPMD.

Strategy: dst-shard the edge aggregation. Host sorts edges by dst and packs
them into fixed-size chunks of 128 edges grouped under 128-node dst blocks.
Each core owns a contiguous range of dst blocks and computes output rows for
its own nodes entirely locally; the only collective is a small AllGather of
the layer-1 node table between the two GAT layers.

Per-edge work is expressed with one-hot selection matrices (S[e, j] =
[dst_rel[e] == j]) built on the vector engine, turning segment softmax +
scatter-add into PE matmuls that accumulate numerator and denominator in
PSUM in a single pass. Softmax max-subtraction is skipped (logits are O(10),
exp stays in fp32 range; identical result up to fp rounding).
"""

import numpy as np

import concourse.bacc as bacc
import concourse.bass as bass
import concourse.mybir as mybir
import concourse.tile as tile
from concourse.bass_utils import run_bass_kernel_spmd
from bass_rust import add_dep_helper


def _dep(a, b, reason):
    ia = a.ins if hasattr(a, "ins") else a
    ib = b.ins if hasattr(b, "ins") else b
    add_dep_helper(ia, ib, reason=reason)

P = 128
NCORES = 8
EPS = 1e-16
NEG_SLOPE = 0.2
F32 = mybir.dt.float32
I32 = mybir.dt.int32


class Cfg:
    def __init__(self, n_nodes, m_chunks, c_in=128, h1=8, ch1=32, c2=64,
                 ncores=NCORES):
        self.n = n_nodes
        self.c_in = c_in
        self.h1 = h1          # heads layer 1
        self.ch1 = ch1        # channels per head layer 1
        self.hc1 = h1 * ch1   # 256
        self.c2 = c2          # layer-2 width (1 head)
        self.ncores = ncores
        self.bpc = -(-n_nodes // (P * ncores))   # blocks per core
        self.npad = ncores * self.bpc * P
        self.nblk = ncores * self.bpc
        self.m = m_chunks                         # chunks (of 128 edges) per block
        # table row widths
        self.t1w = self.hc1 + self.h1            # [h1 | a_src1]  = 264
        self.t2w = self.c2 + 1                   # [h2 | a_src2]  = 65


def host_prep(cfg, edge_index):
    """Sort edges by dst, bucket into 128-node blocks, pad each block to a
    uniform number M of 128-edge chunks. Returns per-core index arrays."""
    n = cfg.n
    src = np.asarray(edge_index[0]).astype(np.int64)
    dst = np.asarray(edge_index[1]).astype(np.int64)
    loop = np.arange(n, dtype=np.int64)
    src = np.concatenate([src, loop])
    dst = np.concatenate([dst, loop])

    order = np.argsort(dst, kind="stable")
    ss = src[order].astype(np.int32)
    ds = dst[order].astype(np.int32)

    blk = ds >> 7                      # dst block of each edge
    counts = np.bincount(blk, minlength=cfg.nblk)
    m_needed = int(-(-counts.max() // P))
    assert m_needed <= cfg.m, (m_needed, cfg.m)

    starts = np.zeros(cfg.nblk, dtype=np.int64)
    starts[1:] = np.cumsum(counts)[:-1]
    # rank of each edge within its block
    rank = np.arange(ds.size, dtype=np.int64) - starts[blk]
    p = rank % P
    g = rank // P

    srcidx = np.zeros((cfg.nblk, P, cfg.m), dtype=np.int32)
    dstidx = np.zeros((cfg.nblk, P, cfg.m), dtype=np.int32)
    dstrel = np.full((cfg.nblk, P, cfg.m), -1.0, dtype=np.float32)
    srcidx[blk, p, g] = ss
    dstidx[blk, p, g] = ds
    dstrel[blk, p, g] = (ds - (blk << 7)).astype(np.float32)

    bpc = cfg.bpc
    per_core = []
    for c in range(cfg.ncores):
        sl = slice(c * bpc, (c + 1) * bpc)
        per_core.append((srcidx[sl], dstidx[sl], dstrel[sl]))
    return per_core


def build_program(cfg, debug=False):
    nc = bacc.Bacc(None, num_devices=cfg.ncores)
    HC1, H1, CH1, C2 = cfg.hc1, cfg.h1, cfg.ch1, cfg.c2
    T1W, T2W, M, BPC = cfg.t1w, cfg.t2w, cfg.m, cfg.bpc
    NPAD, NBLK = cfg.npad, cfg.nblk

    # ---- I/O ----
    x = nc.dram_tensor("x", [NPAD, cfg.c_in], F32, kind="ExternalInput")
    w1 = nc.dram_tensor("w1", [cfg.c_in, HC1], F32, kind="ExternalInput")
    a1 = nc.dram_tensor("a1", [HC1, 2 * H1], F32, kind="ExternalInput")
    b1b = nc.dram_tensor("b1b", [P, HC1], F32, kind="ExternalInput")
    w2 = nc.dram_tensor("w2", [HC1, C2], F32, kind="ExternalInput")
    a2 = nc.dram_tensor("a2", [C2, 2], F32, kind="ExternalInput")
    b2b = nc.dram_tensor("b2b", [P, C2], F32, kind="ExternalInput")
    iota = nc.dram_tensor("iota", [P, P], F32, kind="ExternalInput")
    ident = nc.dram_tensor("ident", [P, P], F32, kind="ExternalInput")
    srcidx = nc.dram_tensor("srcidx", [BPC, M, P], I32, kind="ExternalInput")
    dstidx = nc.dram_tensor("dstidx", [BPC, M, P], I32, kind="ExternalInput")
    dstrel = nc.dram_tensor("dstrel", [BPC, P, M], F32, kind="ExternalInput")
    out = nc.dram_tensor("out", [BPC * P, C2], F32, kind="ExternalOutput")
    if debug:
        dbg_asrc = nc.dram_tensor("dbg_asrc", [BPC * P, M * H1], F32,
                                  kind="ExternalOutput")
        dbg_adst = nc.dram_tensor("dbg_adst", [BPC * P, M * H1], F32,
                                  kind="ExternalOutput")
        dbg_av = nc.dram_tensor("dbg_av", [BPC * P, M * H1], F32,
                                kind="ExternalOutput")
        dbg_den = nc.dram_tensor("dbg_den", [BPC * P, H1], F32,
                                 kind="ExternalOutput")
        dbg_helu = nc.dram_tensor("dbg_helu", [BPC * P, HC1], F32,
                                  kind="ExternalOutput")

    # ---- internal DRAM ----
    t1main = nc.dram_tensor("t1main", [NPAD, T1W], F32)
    t1adst = nc.dram_tensor("t1adst", [NPAD, H1], F32)
    cc1in = nc.dram_tensor("cc1in", [BPC * P, T2W], F32)
    cc2in = nc.dram_tensor("cc2in", [BPC * P, 1], F32)
    t2main = nc.dram_tensor("t2main", [NPAD, T2W], F32, addr_space="Shared")
    t2adst = nc.dram_tensor("t2adst", [NPAD, 1], F32, addr_space="Shared")

    groups = [list(range(cfg.ncores))]

    with tile.TileContext(nc) as tc:
        with (
            tc.tile_pool(name="const", bufs=1) as cpool,
            tc.tile_pool(name="setup", bufs=1) as spool,
            tc.tile_pool(name="p1", bufs=3) as p1pool,
            tc.tile_pool(name="blk", bufs=2) as bpool,
            tc.tile_pool(name="chunk", bufs=4) as kpool,
            tc.tile_pool(name="post", bufs=2) as opool,
            tc.tile_pool(name="ps", bufs=2, space="PSUM") as ps,
        ):
            # ================= setup: constants & augmented weights ========
            iota_s = cpool.tile([P, P], F32)
            nc.sync.dma_start(out=iota_s[:], in_=iota[:])
            ident_s = cpool.tile([P, P], F32)
            nc.sync.dma_start(out=ident_s[:], in_=ident[:])
            b1b_s = cpool.tile([P, HC1], F32)
            nc.sync.dma_start(out=b1b_s[:], in_=b1b[:])
            b2b_s = cpool.tile([P, C2], F32)
            nc.sync.dma_start(out=b2b_s[:], in_=b2b[:])

            # w1aug = [W1 | W1 @ A1]  : [128, 264+8]
            w1aug = cpool.tile([P, T1W + H1], F32)
            nc.sync.dma_start(out=w1aug[:, 0:HC1], in_=w1[:])
            pW1A = ps.tile([P, 2 * H1], F32, space="PSUM", tag="pacc")
            for gblk in range(HC1 // P):
                pT = ps.tile([P, P], F32, space="PSUM", tag="ptr")
                nc.tensor.transpose(
                    out=pT[:], in_=w1aug[:, gblk * P:(gblk + 1) * P],
                    identity=ident_s[:])
                w1t_g = spool.tile([P, P], F32, tag="w1t")
                nc.scalar.copy(out=w1t_g[:], in_=pT[:])
                a1_g = spool.tile([P, 2 * H1], F32, tag="a1g")
                nc.sync.dma_start(out=a1_g[:], in_=a1[gblk * P:(gblk + 1) * P, :])
                nc.tensor.matmul(out=pW1A[:], lhsT=w1t_g[:], rhs=a1_g[:],
                                 start=(gblk == 0), stop=(gblk == HC1 // P - 1))
            nc.scalar.copy(out=w1aug[:, HC1:HC1 + 2 * H1], in_=pW1A[:])

            # w2aug_g = [W2 | W2 @ A2] per 128-row group of W2 : 2 x [128, 66]
            a2_s = spool.tile([C2, 2], F32)
            nc.sync.dma_start(out=a2_s[:], in_=a2[:])
            w2aug = []
            for gblk in range(HC1 // P):
                wg = cpool.tile([P, C2 + 2], F32, tag=f"w2aug{gblk}")
                nc.sync.dma_start(out=wg[:, 0:C2],
                                  in_=w2[gblk * P:(gblk + 1) * P, :])
                pT = ps.tile([C2, P], F32, space="PSUM", tag="ptr")
                nc.tensor.transpose(out=pT[:], in_=wg[:, 0:C2],
                                    identity=ident_s[:])
                w2t_g = spool.tile([C2, P], F32, tag="w2t")
                nc.scalar.copy(out=w2t_g[:], in_=pT[:])
                pW2A = ps.tile([P, 2], F32, space="PSUM", tag="ph2")
                nc.tensor.matmul(out=pW2A[:], lhsT=w2t_g[:], rhs=a2_s[:],
                                 start=True, stop=True)
                nc.scalar.copy(out=wg[:, C2:C2 + 2], in_=pW2A[:])
                w2aug.append(wg)

            # ================= phase 1: node table (replicated) ============
            # t1main[n] = [h1(n), a_src1(n)], t1adst[n] = a_dst1(n)
            t1_writes = []
            for B in range(NBLK):
                r0 = B * P
                xb = p1pool.tile([P, cfg.c_in], F32, tag="xb")
                nc.sync.dma_start(out=xb[:], in_=x[r0:r0 + P, :])
                pxT = ps.tile([P, P], F32, space="PSUM", tag="ptr")
                nc.tensor.transpose(out=pxT[:], in_=xb[:], identity=ident_s[:])
                xT = p1pool.tile([P, cfg.c_in], F32, tag="xT")
                nc.scalar.copy(out=xT[:], in_=pxT[:])
                ph1 = ps.tile([P, T1W + H1], F32, space="PSUM", tag="pacc")
                nc.tensor.matmul(out=ph1[:], lhsT=xT[:], rhs=w1aug[:],
                                 start=True, stop=True)
                row = p1pool.tile([P, T1W + H1], F32, tag="row")
                nc.scalar.copy(out=row[:], in_=ph1[:])
                t1_writes.append(nc.sync.dma_start(
                    out=t1main[r0:r0 + P, :], in_=row[:, 0:T1W]))
                t1_writes.append(nc.sync.dma_start(
                    out=t1adst[r0:r0 + P, :], in_=row[:, T1W:T1W + H1]))

            # ================= phase 2: layer-1 edge aggregation ===========
            # explicit barrier: all table writes land before any gather
            j1tile = cpool.tile([1, 1], F32, tag="j1")
            j1 = nc.gpsimd.memset(j1tile[:], 0.0)
            for w in t1_writes:
                _dep(j1, w, "gathers wait for node table")
            cc_writes = []
            for b in range(BPC):
                r0 = b * P
                drelb = bpool.tile([P, M], F32, tag="drelb")
                nc.sync.dma_start(out=drelb[:], in_=dstrel[b])
                pmsg = ps.tile([P, HC1], F32, space="PSUM", tag="pacc")
                pden = ps.tile([P, H1], F32, space="PSUM", tag="pden")
                for g in range(M):
                    so_g = kpool.tile([P, 1], I32, tag="so")
                    nc.sync.dma_start(out=so_g[:], in_=srcidx[b, g, :, None])
                    do_g = kpool.tile([P, 1], I32, tag="do")
                    nc.sync.dma_start(out=do_g[:], in_=dstidx[b, g, :, None])
                    gath_g = kpool.tile([P, T1W], F32, tag="gch")
                    g1 = nc.gpsimd.indirect_dma_start(
                        out=gath_g[:], out_offset=None, in_=t1main[:],
                        in_offset=bass.IndirectOffsetOnAxis(ap=so_g[:], axis=0))
                    _dep(g1, j1, "gather after table1")
                    adstg_g = kpool.tile([P, H1], F32, tag="ach")
                    g2 = nc.gpsimd.indirect_dma_start(
                        out=adstg_g[:], out_offset=None, in_=t1adst[:],
                        in_offset=bass.IndirectOffsetOnAxis(ap=do_g[:], axis=0))
                    _dep(g2, j1, "gather after table1")
                    S = kpool.tile([P, P], F32, tag="S")
                    nc.vector.tensor_tensor(
                        out=S[:], in0=iota_s[:],
                        in1=drelb[:, g:g + 1].to_broadcast([P, P]),
                        op=mybir.AluOpType.is_equal)
                    av = kpool.tile([P, H1], F32, tag="av")
                    nc.vector.tensor_tensor(
                        out=av[:], in0=gath_g[:, HC1:T1W], in1=adstg_g[:],
                        op=mybir.AluOpType.add)
                    lk = kpool.tile([P, H1], F32, tag="lk")
                    nc.vector.tensor_scalar_mul(out=lk[:], in0=av[:],
                                                scalar1=NEG_SLOPE)
                    nc.vector.tensor_tensor(out=lk[:], in0=lk[:], in1=av[:],
                                            op=mybir.AluOpType.max)
                    expe = kpool.tile([P, H1], F32, tag="expe")
                    nc.scalar.activation(
                        out=expe[:], in_=lk[:],
                        func=mybir.ActivationFunctionType.Exp)
                    if debug:
                        nc.sync.dma_start(
                            out=dbg_av[r0:r0 + P, g * H1:(g + 1) * H1],
                            in_=av[:])
                    wm = kpool.tile([P, HC1], F32, tag="wm")
                    for h in range(H1):
                        nc.vector.tensor_scalar_mul(
                            out=wm[:, h * CH1:(h + 1) * CH1],
                            in0=gath_g[:, h * CH1:(h + 1) * CH1],
                            scalar1=expe[:, h:h + 1])
                    nc.tensor.matmul(out=pmsg[:], lhsT=S[:], rhs=wm[:],
                                     start=(g == 0), stop=(g == M - 1))
                    nc.tensor.matmul(out=pden[:], lhsT=S[:], rhs=expe[:],
                                     start=(g == 0), stop=(g == M - 1))

                # ---- finalize block: softmax divide, bias, ELU ----
                den = opool.tile([P, H1], F32, tag="den")
                nc.vector.tensor_scalar_add(out=den[:], in0=pden[:], scalar1=EPS)
                if debug:
                    nc.sync.dma_start(out=dbg_den[r0:r0 + P, :], in_=den[:])
                rec = opool.tile([P, H1], F32, tag="rec")
                nc.vector.reciprocal(out=rec[:], in_=den[:])
                o1 = opool.tile([P, HC1], F32, tag="o1")
                for h in range(H1):
                    nc.vector.tensor_scalar_mul(
                        out=o1[:, h * CH1:(h + 1) * CH1],
                        in0=pmsg[:, h * CH1:(h + 1) * CH1],
                        scalar1=rec[:, h:h + 1])
                o1b = opool.tile([P, HC1], F32, tag="o1b")
                nc.vector.tensor_tensor(out=o1b[:], in0=o1[:], in1=b1b_s[:],
                                        op=mybir.AluOpType.add)
                # elu(x) = relu(x) + exp(min(x,0)) - 1
                xneg = opool.tile([P, HC1], F32, tag="xneg")
                nc.vector.tensor_scalar_min(out=xneg[:], in0=o1b[:], scalar1=0.0)
                en = opool.tile([P, HC1], F32, tag="en")
                nc.scalar.activation(out=en[:], in_=xneg[:],
                                     func=mybir.ActivationFunctionType.Exp)
                xpos = opool.tile([P, HC1], F32, tag="xpos")
                nc.scalar.activation(out=xpos[:], in_=o1b[:],
                                     func=mybir.ActivationFunctionType.Relu)
                helu = opool.tile([P, HC1], F32, tag="helu")
                nc.vector.tensor_tensor(out=helu[:], in0=xpos[:], in1=en[:],
                                        op=mybir.AluOpType.add)
                nc.vector.tensor_scalar_add(out=helu[:], in0=helu[:],
                                            scalar1=-1.0)

                if debug:
                    nc.sync.dma_start(out=dbg_helu[r0:r0 + P, :], in_=helu[:])
                # ---- h2aug = helu @ w2aug ----
                ph2 = ps.tile([P, T2W + 1], F32, space="PSUM", tag="ph2")
                for gblk in range(HC1 // P):
                    phT = ps.tile([P, P], F32, space="PSUM", tag="ptr")
                    nc.tensor.transpose(
                        out=phT[:], in_=helu[:, gblk * P:(gblk + 1) * P],
                        identity=ident_s[:])
                    hT = opool.tile([P, P], F32, tag="hT")
                    nc.scalar.copy(out=hT[:], in_=phT[:])
                    nc.tensor.matmul(out=ph2[:], lhsT=hT[:],
                                     rhs=w2aug[gblk][:],
                                     start=(gblk == 0),
                                     stop=(gblk == HC1 // P - 1))
                h2row = opool.tile([P, T2W + 1], F32, tag="h2row")
                nc.scalar.copy(out=h2row[:], in_=ph2[:])
                cc_writes.append(nc.sync.dma_start(
                    out=cc1in[r0:r0 + P, :], in_=h2row[:, 0:T2W]))
                cc_writes.append(nc.sync.dma_start(
                    out=cc2in[r0:r0 + P, :], in_=h2row[:, T2W:T2W + 1]))

            # ================= phase 3: share layer-2 node table ===========
            cc1 = nc.gpsimd.collective_compute(
                "AllGather", mybir.AluOpType.bypass, replica_groups=groups,
                ins=[cc1in[:]], outs=[t2main[:]])
            cc2 = nc.gpsimd.collective_compute(
                "AllGather", mybir.AluOpType.bypass, replica_groups=groups,
                ins=[cc2in[:]], outs=[t2adst[:]])
            for w in cc_writes:
                _dep(cc1, w, "allgather after cc writes")
                _dep(cc2, w, "allgather after cc writes")
            j2tile = cpool.tile([1, 1], F32, tag="j2")
            j2 = nc.gpsimd.memset(j2tile[:], 0.0)
            _dep(j2, cc1, "phase4 after allgather")
            _dep(j2, cc2, "phase4 after allgather")

            # ================= phase 4: layer-2 edge aggregation ===========
            for b in range(BPC):
                r0 = b * P
                drelb = bpool.tile([P, M], F32, tag="drelb")
                nc.sync.dma_start(out=drelb[:], in_=dstrel[b])
                pmsg2 = ps.tile([P, C2], F32, space="PSUM", tag="pacc")
                pden2 = ps.tile([P, 1], F32, space="PSUM", tag="pden")
                for g in range(M):
                    so_g = kpool.tile([P, 1], I32, tag="so")
                    nc.sync.dma_start(out=so_g[:], in_=srcidx[b, g, :, None])
                    do_g = kpool.tile([P, 1], I32, tag="do")
                    nc.sync.dma_start(out=do_g[:], in_=dstidx[b, g, :, None])
                    gath2_g = kpool.tile([P, T2W], F32, tag="gch2")
                    g3 = nc.gpsimd.indirect_dma_start(
                        out=gath2_g[:], out_offset=None, in_=t2main[:],
                        in_offset=bass.IndirectOffsetOnAxis(ap=so_g[:], axis=0))
                    _dep(g3, j2, "gather after table2")
                    adst2g_g = kpool.tile([P, 1], F32, tag="ach2")
                    g4 = nc.gpsimd.indirect_dma_start(
                        out=adst2g_g[:], out_offset=None, in_=t2adst[:],
                        in_offset=bass.IndirectOffsetOnAxis(ap=do_g[:], axis=0))
                    _dep(g4, j2, "gather after table2")
                    S = kpool.tile([P, P], F32, tag="S")
                    nc.vector.tensor_tensor(
                        out=S[:], in0=iota_s[:],
                        in1=drelb[:, g:g + 1].to_broadcast([P, P]),
                        op=mybir.AluOpType.is_equal)
                    av = kpool.tile([P, 1], F32, tag="av2")
                    nc.vector.tensor_tensor(
                        out=av[:], in0=gath2_g[:, C2:T2W], in1=adst2g_g[:],
                        op=mybir.AluOpType.add)
                    lk = kpool.tile([P, 1], F32, tag="lk2")
                    nc.vector.tensor_scalar_mul(out=lk[:], in0=av[:],
                                                scalar1=NEG_SLOPE)
                    nc.vector.tensor_tensor(out=lk[:], in0=lk[:], in1=av[:],
                                            op=mybir.AluOpType.max)
                    expe = kpool.tile([P, 1], F32, tag="expe2")
                    nc.scalar.activation(
                        out=expe[:], in_=lk[:],
                        func=mybir.ActivationFunctionType.Exp)
                    wm = kpool.tile([P, C2], F32, tag="wm2")
                    nc.vector.tensor_scalar_mul(out=wm[:], in0=gath2_g[:, 0:C2],
                                                scalar1=expe[:, 0:1])
                    nc.tensor.matmul(out=pmsg2[:], lhsT=S[:], rhs=wm[:],
                                     start=(g == 0), stop=(g == M - 1))
                    nc.tensor.matmul(out=pden2[:], lhsT=S[:], rhs=expe[:],
                                     start=(g == 0), stop=(g == M - 1))

                den2 = opool.tile([P, 1], F32, tag="den2")
                nc.vector.tensor_scalar_add(out=den2[:], in0=pden2[:],
                                            scalar1=EPS)
                rec2 = opool.tile([P, 1], F32, tag="rec2")
                nc.vector.reciprocal(out=rec2[:], in_=den2[:])
                o2 = opool.tile([P, C2], F32, tag="o2")
                nc.vector.tensor_scalar_mul(out=o2[:], in0=pmsg2[:],
                                            scalar1=rec2[:, 0:1])
                o2b = opool.tile([P, C2], F32, tag="o2b")
                nc.vector.tensor_tensor(out=o2b[:], in0=o2[:], in1=b2b_s[:],
                                        op=mybir.AluOpType.add)
                nc.sync.dma_start(out=out[r0:r0 + P, :], in_=o2b[:])

    nc.compile()
    return nc


def make_in_maps(cfg, x, W1, att_src1, att_dst1, bias1, W2, att_src2,
                 att_dst2, bias2, edge_index):
    H1, CH1, HC1, C2 = cfg.h1, cfg.ch1, cfg.hc1, cfg.c2
    x = np.asarray(x, dtype=np.float32)
    xpad = np.zeros((cfg.npad, cfg.c_in), dtype=np.float32)
    xpad[: cfg.n] = x

    A1 = np.zeros((HC1, 2 * H1), dtype=np.float32)
    hh = np.repeat(np.arange(H1), CH1)
    A1[np.arange(HC1), hh] = np.asarray(att_src1, np.float32).reshape(-1)
    A1[np.arange(HC1), H1 + hh] = np.asarray(att_dst1, np.float32).reshape(-1)
    A2 = np.stack([np.asarray(att_src2, np.float32).reshape(-1),
                   np.asarray(att_dst2, np.float32).reshape(-1)], axis=1)

    b1b = np.tile(np.asarray(bias1, np.float32).reshape(1, -1), (P, 1))
    b2b = np.tile(np.asarray(bias2, np.float32).reshape(1, -1), (P, 1))
    iota = np.tile(np.arange(P, dtype=np.float32), (P, 1))
    ident = np.eye(P, dtype=np.float32)

    per_core = host_prep(cfg, edge_index)
    shared = {
        "x": xpad, "w1": np.asarray(W1, np.float32), "a1": A1, "b1b": b1b,
        "w2": np.asarray(W2, np.float32), "a2": A2, "b2b": b2b,
        "iota": iota, "ident": ident,
    }
    in_maps = []
    for c in range(cfg.ncores):
        si, di, dr = per_core[c]
        m = dict(shared)
        m["srcidx"] = np.ascontiguousarray(si.transpose(0, 2, 1))
        m["dstidx"] = np.ascontiguousarray(di.transpose(0, 2, 1))
        m["dstrel"] = np.ascontiguousarray(dr)
        in_maps.append(m)
    return in_maps


_prog_cache = {}
_last_results = None


def kernel(x, edge_index, edge_weight, W1, att_src1, att_dst1, bias1,
           W2, att_src2, att_dst2, bias2):
    global _last_results
    n = x.shape[0]
    # edge_weight is unused (GATConv with edge_dim=None ignores it)
    # chunk capacity: computed from data, padded up for program stability
    src = np.asarray(edge_index[1]).astype(np.int64)
    dst = np.concatenate([src, np.arange(n, dtype=np.int64)])
    counts = np.bincount(dst >> 7, minlength=-(-n // P))
    m_needed = int(-(-counts.max() // P))
    m = max(20, m_needed)

    cfg = Cfg(n, m, c_in=x.shape[1], h1=8, ch1=32, c2=64)
    key = (cfg.n, cfg.c_in, cfg.m)
    if key not in _prog_cache:
        _prog_cache[key] = build_program(cfg)
    nc = _prog_cache[key]

    in_maps = make_in_maps(cfg, x, W1, att_src1, att_dst1, bias1, W2,
                           att_src2, att_dst2, bias2, edge_index)
    res = run_bass_kernel_spmd(nc, in_maps, list(range(cfg.ncores)))
    _last_results = res
    outs = [res.results[c]["out"] for c in range(cfg.ncores)]
    full = np.concatenate(outs, axis=0)[: cfg.n]
    return np.ascontiguousarray(full)



# revision 18
# speedup vs baseline: 1065.0067x; 1065.0067x over previous
"""GAT (2-layer, PyG-style) Trainium2 Bass kernel, 8-core SPMD. v3.

Strategy (dst-sharded edge aggregation, fp16 edge path):
- Host sorts edges by dst into 128-node blocks; within a block edges are
  ordered [src<32768 | src>=32768], each group padded to a multiple of 128
  (MLO/MHI chunks of 128 edge slots, global maxima). Core c owns blocks
  [c*BPC, (c+1)*BPC) and produces output rows for its own nodes only.
- Node tables live in DRAM with rows padded to 256B multiples so that
  InstDMAGatherAnt (int16 indices, one descriptor per edge, ~1us fixed cost
  per call) batches a whole block's gathers into 2 calls (low/high rows).
- a_dst is gathered from small per-core local tables whose indices fit
  int16: t1loc (written by phase 0) for layer 1, cc_in itself for layer 2.
- Edge softmax + scatter-add are expressed via one-hot S matrices + PE
  matmuls (fp16 in, fp32 PSUM accumulate). Logits are exp-shifted by -2
  (cancels exactly in softmax; keeps fp16 exp finite).
- ELU's "-1" is dropped (eluplus = relu(x)+exp(min(x,0))) and corrected at
  the end: out -= colsum(W2) (exact since softmax weights sum to 1); the
  induced constant layer-2 logit shift c0 is subtracted pre-leaky_relu.
- One AllGather of the 128-col fp16 layer-2 table is the only collective.
"""

import numpy as np

import concourse.bacc as bacc
import concourse.bass as bass
import concourse.mybir as mybir
import concourse.tile as tile
from concourse import library_config
from concourse.bass_utils import run_bass_kernel_spmd
from bass_rust import add_dep_helper


def _dep(a, b, reason):
    ia = a.ins if hasattr(a, "ins") else a
    ib = b.ins if hasattr(b, "ins") else b
    add_dep_helper(ia, ib, reason=reason)


P = 128
NCORES = 8
EPS = 1e-16
NEG_SLOPE = 0.2
LO = 32768                  # int16 index limit -> low/high table split
F32 = mybir.dt.float32
F16 = mybir.dt.float16
I32 = mybir.dt.int32
I16 = mybir.dt.int16
AF = mybir.ActivationFunctionType
ALU = mybir.AluOpType


class Cfg:
    def __init__(self, n_nodes, mlo, mhi, c_in=128, h1=8, ch1=32, c2=64,
                 ncores=NCORES):
        self.n = n_nodes
        self.c_in = c_in
        self.h1 = h1
        self.ch1 = ch1
        self.hc1 = h1 * ch1          # 256
        self.c2 = c2
        self.ncores = ncores
        self.bpc = -(-n_nodes // (P * ncores))      # 49
        self.npad = ncores * self.bpc * P
        self.nblk = ncores * self.bpc
        self.mlo = mlo
        self.mhi = mhi
        self.mb = mlo + mhi
        self.t1w = 384               # [h 256 | asrc 8 | adst 8 | pad]
        self.t2w = 128               # [h2 64 | asrc2 1 | adst2 1 | pad]
        self.nloc = ((self.bpc + 7) // 8) * 8 * P   # t1loc rows
        # eidx blob layout per block, in i32 columns:
        self.o_lo = 0                          # srclo idx16: mlo*4 i32 cols
        self.o_hi = self.o_lo + mlo * 4        # srchi idx16: mhi*4
        self.o_ad = self.o_hi + mhi * 4        # adst idx16:  mb*4
        self.o_dr = self.o_ad + self.mb * 4    # drel f32:    mb
        self.K = self.o_dr + self.mb



def _gpieces(nc, dep_fn, out_tile, cbase, W, in_ap, idx16, nchunks, dep, why):
    """Emit dma_gathers in <=512-index pieces (4 chunks of 128)."""
    for k0 in range(0, nchunks, 4):
        nk = min(4, nchunks - k0)
        g = nc.gpsimd.dma_gather(
            out_ap=out_tile[:, (cbase + k0) * W:(cbase + k0 + nk) * W]
                .rearrange("p (m w) -> p m w", m=nk),
            in_ap=in_ap, idxs_ap=idx16[:, k0 * 8:(k0 + nk) * 8],
            num_idxs=nk * 128, num_idxs_reg=nk * 128, elem_size=W)
        dep_fn(g, dep, why)


def build_program(cfg):
    nc = bacc.Bacc(None, num_devices=cfg.ncores)
    HC1, H1, CH1, C2 = cfg.hc1, cfg.h1, cfg.ch1, cfg.c2
    T1W, T2W, BPC = cfg.t1w, cfg.t2w, cfg.bpc
    MLO, MHI, MB = cfg.mlo, cfg.mhi, cfg.mb
    NBLK, NPAD, NLOC = cfg.nblk, cfg.npad, cfg.nloc
    G1 = 8
    assert NBLK % G1 == 0
    G0 = NLOC // (G1 * P)

    # ---- I/O ----
    xt = nc.dram_tensor("xt", [cfg.c_in, NPAD], F16, kind="ExternalInput")
    w1aug = nc.dram_tensor("w1aug", [cfg.c_in, 272], F16, kind="ExternalInput")
    w2aug = nc.dram_tensor("w2aug", [HC1, 66], F16, kind="ExternalInput")
    b1b = nc.dram_tensor("b1b", [P, HC1], F16, kind="ExternalInput")
    b2b = nc.dram_tensor("b2b", [P, C2], F32, kind="ExternalInput")
    iota = nc.dram_tensor("iota", [P, P], F16, kind="ExternalInput")
    ident = nc.dram_tensor("ident", [P, P], F16, kind="ExternalInput")
    shifts = nc.dram_tensor("shifts", [P, 2], F32, kind="ExternalInput")
    xoff = nc.dram_tensor("xoff", [P, G0], I32, kind="ExternalInput")
    eidx = nc.dram_tensor("eidx", [BPC, P, cfg.K], I32, kind="ExternalInput")
    out = nc.dram_tensor("out", [BPC * P, C2], F32, kind="ExternalOutput")

    # ---- internal DRAM ----
    t1 = nc.dram_tensor("t1", [NPAD, T1W], F16)
    t1loc = nc.dram_tensor("t1loc", [NLOC, T2W], F16)
    cc_in = nc.dram_tensor("cc_in", [BPC * P, T2W], F16)
    t2 = nc.dram_tensor("t2", [NPAD, T2W], F16, addr_space="Shared")

    groups = [list(range(cfg.ncores))]

    with tile.TileContext(nc) as tc:
        with (
            tc.tile_pool(name="const", bufs=1) as cpool,
            tc.tile_pool(name="p1", bufs=2) as p1pool,
            tc.tile_pool(name="blk", bufs=2) as bpool,
            tc.tile_pool(name="s", bufs=4) as spool,
            tc.tile_pool(name="fin", bufs=2) as opool,
            tc.tile_pool(name="ps", bufs=2, space="PSUM") as ps,
        ):
            # ---------------- constants ----------------
            iota_s = cpool.tile([P, P], F16)
            nc.sync.dma_start(out=iota_s[:], in_=iota[:])
            ident_s = cpool.tile([P, P], F16)
            nc.sync.dma_start(out=ident_s[:], in_=ident[:])
            w1aug_s = cpool.tile([P, 272], F16)
            nc.sync.dma_start(out=w1aug_s[:], in_=w1aug[:])
            w2aug_s = []
            for j in range(HC1 // P):
                wg = cpool.tile([P, 66], F16, tag=f"w2aug{j}")
                nc.sync.dma_start(out=wg[:], in_=w2aug[j * P:(j + 1) * P, :])
                w2aug_s.append(wg)
            b1b_s = cpool.tile([P, HC1], F16)
            nc.sync.dma_start(out=b1b_s[:], in_=b1b[:])
            b2b_s = cpool.tile([P, C2], F32)
            nc.sync.dma_start(out=b2b_s[:], in_=b2b[:])
            shifts_s = cpool.tile([P, 2], F32)
            nc.sync.dma_start(out=shifts_s[:], in_=shifts[:])
            xoff_s = cpool.tile([P, G0], I32)
            nc.sync.dma_start(out=xoff_s[:], in_=xoff[:])

            # ------------- phase 0: local a_dst table (own blocks) -------
            loc_writes = []
            for g in range(G0):
                xg0 = p1pool.tile([P, G1 * P], F16, tag="xg0")
                nc.gpsimd.indirect_dma_start(
                    out=xg0[:], out_offset=None, in_=xt[:],
                    in_offset=bass.IndirectOffsetOnAxis(
                        ap=xoff_s[:, g:g + 1], axis=1))
                rows0 = p1pool.tile([P, G1 * H1], F16, tag="rows0")
                for j in range(G1):
                    pl = ps.tile([P, H1], F32, space="PSUM", tag="den")
                    nc.tensor.matmul(out=pl[:], lhsT=xg0[:, j * P:(j + 1) * P],
                                     rhs=w1aug_s[:, 264:272],
                                     start=True, stop=True)
                    nc.vector.tensor_scalar_add(
                        out=rows0[:, j * H1:(j + 1) * H1], in0=pl[:],
                        scalar1=0.0)
                loc_writes.append(nc.sync.dma_start(
                    out=t1loc[g * G1 * P:(g + 1) * G1 * P, 0:H1].rearrange(
                        "(j p) c -> p j c", j=G1),
                    in_=rows0[:].rearrange("p (j c) -> p j c", j=G1)))
            jloc_t = cpool.tile([1, 1], F32, tag="jloc")
            jloc = nc.gpsimd.memset(jloc_t[:], 0.0)
            for w in loc_writes:
                _dep(jloc, w, "adst gathers wait for local table")

            # ------------- phase 1: full node table (replicated) ---------
            t1_writes = []
            for grp in range(NBLK // G1):
                B0 = grp * G1
                xg = p1pool.tile([P, G1 * P], F16, tag="xg")
                nc.sync.dma_start(out=xg[:], in_=xt[:, B0 * P:(B0 + G1) * P])
                rows = p1pool.tile([P, G1 * 272], F16, tag="rows")
                for j in range(G1):
                    ph1 = ps.tile([P, 272], F32, space="PSUM", tag="acc")
                    nc.tensor.matmul(out=ph1[:], lhsT=xg[:, j * P:(j + 1) * P],
                                     rhs=w1aug_s[:], start=True, stop=True)
                    dst = rows[:, j * 272:(j + 1) * 272]
                    if j % 2 == 0:
                        nc.scalar.copy(out=dst, in_=ph1[:])
                    else:
                        nc.vector.tensor_scalar_add(out=dst, in0=ph1[:],
                                                    scalar1=0.0)
                t1_writes.append(nc.sync.dma_start(
                    out=t1[B0 * P:(B0 + G1) * P, 0:272].rearrange(
                        "(j p) c -> p j c", j=G1),
                    in_=rows[:].rearrange("p (j c) -> p j c", j=G1)))

            j1tile = cpool.tile([1, 1], F32, tag="j1")
            j1 = nc.gpsimd.memset(j1tile[:], 0.0)
            for w in t1_writes:
                _dep(j1, w, "layer1 gathers wait for full node table")

            # gpsimd ext-isa library containing InstDMAGatherAnt
            nc.gpsimd.load_library(library_config.mlp)

            # ------------- phase 2: layer-1 edge aggregation -------------
            cc_writes = []
            for b in range(BPC):
                r0 = b * P
                tlo = bpool.tile([P, MLO * 4], I32, tag="tlo")
                nc.sync.dma_start(out=tlo[:], in_=eidx[b, :, cfg.o_lo:cfg.o_hi])
                thi = bpool.tile([P, MHI * 4], I32, tag="thi")
                nc.sync.dma_start(out=thi[:], in_=eidx[b, :, cfg.o_hi:cfg.o_ad])
                tad = bpool.tile([P, MB * 4], I32, tag="tad")
                nc.sync.dma_start(out=tad[:], in_=eidx[b, :, cfg.o_ad:cfg.o_dr])
                tdr = bpool.tile([P, MB], I32, tag="tdr")
                nc.sync.dma_start(out=tdr[:], in_=eidx[b, :, cfg.o_dr:cfg.K])
                ilo = tlo[:].bitcast(I16)
                ihi = thi[:].bitcast(I16)
                iad = tad[:].bitcast(I16)
                drel = tdr[:].bitcast(F32)

                gath = bpool.tile([P, MB * T1W], F16, tag="gath")
                _gpieces(nc, _dep, gath, 0, T1W, t1[0:LO, :], ilo, MLO,
                         j1, "lo gather after table1")
                _gpieces(nc, _dep, gath, MLO, T1W, t1[LO:NPAD, :], ihi, MHI,
                         j1, "hi gather after table1")
                adstg = bpool.tile([P, MB * T2W], F16, tag="adstg")
                _gpieces(nc, _dep, adstg, 0, T2W, t1loc[:], iad, MB,
                         jloc, "adst gather after local table")

                gv = gath[:].rearrange("p (m w) -> p m w", m=MB)
                av = bpool.tile([P, MB * H1], F16, tag="av")
                nc.vector.tensor_tensor(
                    out=av[:].rearrange("p (m h) -> p m h", m=MB),
                    in0=gv[:, :, 256:264],
                    in1=adstg[:].rearrange("p (m w) -> p m w",
                                           m=MB)[:, :, 0:H1],
                    op=ALU.add)
                lk = bpool.tile([P, MB * H1], F16, tag="lk")
                nc.vector.scalar_tensor_tensor(
                    out=lk[:], in0=av[:], scalar=NEG_SLOPE, in1=av[:],
                    op0=ALU.mult, op1=ALU.max)
                efull = bpool.tile([P, MB * HC1], F16, tag="efull")
                nc.scalar.activation(
                    out=efull[:],
                    in_=lk[:].rearrange("p (m h) -> p m h", m=MB)
                          .to_broadcast([P, MB, H1, CH1]),
                    func=AF.Exp, bias=shifts_s[:, 0:1])
                wm = bpool.tile([P, MB * HC1], F16, tag="wm")
                nc.vector.tensor_tensor(
                    out=wm[:].rearrange("p (m c) -> p m c", m=MB),
                    in0=gv[:, :, 0:HC1],
                    in1=efull[:].rearrange("p (m c) -> p m c", m=MB),
                    op=ALU.mult)

                pacc = ps.tile([P, 272], F32, space="PSUM", tag="acc")
                pmsg = pacc[:, 0:HC1]
                pdent = ps.tile([P, H1], F32, space="PSUM", tag="den")
                pden = pdent[:]
                e4 = efull[:].rearrange("p (m h c) -> p m h c", m=MB, h=H1)
                for g in range(MB):
                    S = spool.tile([P, P], F16, tag="S")
                    nc.vector.tensor_scalar(
                        out=S[:], in0=iota_s[:], scalar1=drel[:, g:g + 1],
                        scalar2=None, op0=ALU.is_equal)
                    nc.tensor.matmul(out=pmsg, lhsT=S[:],
                                     rhs=wm[:, g * HC1:(g + 1) * HC1],
                                     start=(g == 0), stop=(g == MB - 1))
                    nc.tensor.matmul(out=pden, lhsT=S[:],
                                     rhs=e4[:, g, :, 0],
                                     start=(g == 0), stop=(g == MB - 1))

                den = opool.tile([P, H1], F32, tag="den")
                nc.vector.tensor_scalar_add(out=den[:], in0=pden,
                                            scalar1=EPS)
                rec = opool.tile([P, H1], F32, tag="rec")
                nc.vector.reciprocal(out=rec[:], in_=den[:])
                o1b = opool.tile([P, HC1], F16, tag="o1b")
                for h in range(H1):
                    sl = slice(h * CH1, (h + 1) * CH1)
                    nc.vector.scalar_tensor_tensor(
                        out=o1b[:, sl], in0=pacc[:, sl],
                        scalar=rec[:, h:h + 1], in1=b1b_s[:, sl],
                        op0=ALU.mult, op1=ALU.add)
                xn = opool.tile([P, HC1], F16, tag="xn")
                nc.gpsimd.tensor_scalar_min(out=xn[:], in0=o1b[:], scalar1=0.0)
                en = opool.tile([P, HC1], F16, tag="en")
                nc.scalar.activation(out=en[:], in_=xn[:], func=AF.Exp)
                helu = opool.tile([P, HC1], F16, tag="helu")
                nc.vector.scalar_tensor_tensor(
                    out=helu[:], in0=o1b[:], scalar=0.0, in1=en[:],
                    op0=ALU.max, op1=ALU.add)

                ph2 = ps.tile([P, 66], F32, space="PSUM", tag="ph2")
                for j in range(HC1 // P):
                    pT = ps.tile([P, P], F16, space="PSUM", tag="pT")
                    nc.tensor.transpose(out=pT[:],
                                        in_=helu[:, j * P:(j + 1) * P],
                                        identity=ident_s[:])
                    hT = opool.tile([P, P], F16, tag="hT")
                    nc.vector.tensor_scalar_add(out=hT[:], in0=pT[:],
                                                scalar1=0.0)
                    nc.tensor.matmul(out=ph2[:], lhsT=hT[:], rhs=w2aug_s[j][:],
                                     start=(j == 0), stop=(j == HC1 // P - 1))
                h2row = opool.tile([P, T2W], F16, tag="h2row")
                nc.vector.tensor_scalar_add(out=h2row[:, 0:66], in0=ph2[:],
                                            scalar1=0.0)
                nc.vector.memset(h2row[:, 66:T2W], 0.0)
                cc_writes.append(nc.sync.dma_start(
                    out=cc_in[r0:r0 + P, :], in_=h2row[:]))

            # ------------- phase 3: share layer-2 node table -------------
            nc.gpsimd.load_library(library_config.standard)
            cc = nc.gpsimd.collective_compute(
                "AllGather", ALU.bypass, replica_groups=groups,
                ins=[cc_in[:]], outs=[t2[:]])
            for w in cc_writes:
                _dep(cc, w, "allgather after cc writes")
            j2tile = cpool.tile([1, 1], F32, tag="j2")
            j2 = nc.gpsimd.memset(j2tile[:], 0.0)
            _dep(j2, cc, "layer2 gathers after allgather")
            nc.gpsimd.load_library(library_config.mlp)

            # ------------- phase 4: layer-2 edge aggregation -------------
            for b in range(BPC):
                r0 = b * P
                tlo = bpool.tile([P, MLO * 4], I32, tag="tlo2")
                nc.sync.dma_start(out=tlo[:], in_=eidx[b, :, cfg.o_lo:cfg.o_hi])
                thi = bpool.tile([P, MHI * 4], I32, tag="thi2")
                nc.sync.dma_start(out=thi[:], in_=eidx[b, :, cfg.o_hi:cfg.o_ad])
                tad = bpool.tile([P, MB * 4], I32, tag="tad2")
                nc.sync.dma_start(out=tad[:], in_=eidx[b, :, cfg.o_ad:cfg.o_dr])
                tdr = bpool.tile([P, MB], I32, tag="tdr2")
                nc.sync.dma_start(out=tdr[:], in_=eidx[b, :, cfg.o_dr:cfg.K])
                ilo = tlo[:].bitcast(I16)
                ihi = thi[:].bitcast(I16)
                iad = tad[:].bitcast(I16)
                drel = tdr[:].bitcast(F32)

                gath2 = bpool.tile([P, MB * T2W], F16, tag="gath2")
                _gpieces(nc, _dep, gath2, 0, T2W, t2[0:LO, :], ilo, MLO,
                         j2, "lo gather after table2")
                _gpieces(nc, _dep, gath2, MLO, T2W, t2[LO:NPAD, :], ihi, MHI,
                         j2, "hi gather after table2")
                adst2 = bpool.tile([P, MB * T2W], F16, tag="adst2")
                _gpieces(nc, _dep, adst2, 0, T2W, cc_in[:], iad, MB,
                         j2, "adst2 gather after cc writes")

                qv = gath2[:].rearrange("p (m w) -> p m w", m=MB)
                av2 = bpool.tile([P, MB], F16, tag="av2")
                nc.vector.scalar_tensor_tensor(
                    out=av2[:].rearrange("p (m o) -> p m o", m=MB),
                    in0=qv[:, :, 64:65], scalar=shifts_s[:, 1:2],
                    in1=adst2[:].rearrange("p (m w) -> p m w",
                                           m=MB)[:, :, 65:66],
                    op0=ALU.add, op1=ALU.add)
                lk2 = bpool.tile([P, MB], F16, tag="lk2")
                nc.vector.scalar_tensor_tensor(
                    out=lk2[:], in0=av2[:], scalar=NEG_SLOPE, in1=av2[:],
                    op0=ALU.mult, op1=ALU.max)
                e2full = bpool.tile([P, MB * C2], F16, tag="e2full")
                nc.scalar.activation(
                    out=e2full[:],
                    in_=lk2[:].rearrange("p (m o) -> p m o", m=MB)
                           .to_broadcast([P, MB, 1, C2]),
                    func=AF.Exp, bias=shifts_s[:, 0:1])
                wm2 = bpool.tile([P, MB * C2], F16, tag="wm2")
                nc.vector.tensor_tensor(
                    out=wm2[:].rearrange("p (m c) -> p m c", m=MB),
                    in0=qv[:, :, 0:C2],
                    in1=e2full[:].rearrange("p (m c) -> p m c", m=MB),
                    op=ALU.mult)

                pacc2 = ps.tile([P, 272], F32, space="PSUM", tag="acc")
                pmsg2 = pacc2[:, 0:C2]
                pdent2 = ps.tile([P, H1], F32, space="PSUM", tag="den")
                pden2 = pdent2[:, 0:1]
                for g in range(MB):
                    S = spool.tile([P, P], F16, tag="S")
                    nc.vector.tensor_scalar(
                        out=S[:], in0=iota_s[:], scalar1=drel[:, g:g + 1],
                        scalar2=None, op0=ALU.is_equal)
                    nc.tensor.matmul(out=pmsg2, lhsT=S[:],
                                     rhs=wm2[:, g * C2:(g + 1) * C2],
                                     start=(g == 0), stop=(g == MB - 1))
                    nc.tensor.matmul(out=pden2, lhsT=S[:],
                                     rhs=e2full[:, g * C2:g * C2 + 1],
                                     start=(g == 0), stop=(g == MB - 1))

                den2 = opool.tile([P, 1], F32, tag="den2")
                nc.vector.tensor_scalar_add(out=den2[:], in0=pden2,
                                            scalar1=EPS)
                rec2 = opool.tile([P, 1], F32, tag="rec2")
                nc.vector.reciprocal(out=rec2[:], in_=den2[:])
                o2 = opool.tile([P, C2], F32, tag="o2")
                nc.vector.scalar_tensor_tensor(
                    out=o2[:], in0=pmsg2, scalar=rec2[:, 0:1],
                    in1=b2b_s[:], op0=ALU.mult, op1=ALU.add)
                nc.sync.dma_start(out=out[r0:r0 + P, :], in_=o2[:])

    nc.compile()
    return nc


def _wrap16(idx, nid):
    """Pack an int16 index list (len nid) into a [128, nid//16] tile:
    element k at (k%16, k//16), replicated to partitions 16..127."""
    a = np.asarray(idx, np.int16).reshape(nid // 16, 16).T  # [16, nid//16]
    return np.tile(a, (8, 1))


def host_prep(cfg, edge_index):
    n = cfg.n
    src = np.asarray(edge_index[0]).astype(np.int64)
    dst = np.asarray(edge_index[1]).astype(np.int64)
    loop = np.arange(n, dtype=np.int64)
    src = np.concatenate([src, loop])
    dst = np.concatenate([dst, loop])

    order = np.argsort(dst, kind="stable")
    ss = src[order]
    ds = dst[order]
    blk = ds >> 7

    MLO, MHI, MB = cfg.mlo, cfg.mhi, cfg.mb
    NBLK, BPC = cfg.nblk, cfg.bpc
    eidx = np.zeros((NBLK, P, cfg.K), dtype=np.int32)

    starts = np.zeros(NBLK + 1, dtype=np.int64)
    np.cumsum(np.bincount(blk, minlength=NBLK), out=starts[1:])

    for B in range(NBLK):
        s_b = ss[starts[B]:starts[B + 1]]
        d_b = ds[starts[B]:starts[B + 1]]
        lo_m = s_b < LO
        s_lo, d_lo = s_b[lo_m], d_b[lo_m]
        s_hi, d_hi = s_b[~lo_m], d_b[~lo_m]
        nlo, nhi = len(s_lo), len(s_hi)
        assert -(-nlo // P) <= MLO and -(-nhi // P) <= MHI, (B, nlo, nhi)
        cbase = (B // BPC) * BPC * P

        ilo = np.zeros(MLO * P, np.int16)
        ilo[:nlo] = s_lo.astype(np.int16)
        ihi = np.zeros(MHI * P, np.int16)
        ihi[:nhi] = (s_hi - LO).astype(np.int16)
        kidx = np.concatenate([np.arange(nlo), MLO * P + np.arange(nhi)])
        d_all = np.concatenate([d_lo, d_hi])
        iad = np.zeros(MB * P, np.int64)
        iad[kidx] = d_all - cbase
        drel = np.full((P, MB), -1.0, dtype=np.float32)
        drel[kidx % P, kidx // P] = (d_all - (B << 7)).astype(np.float32)

        eidx[B, :, cfg.o_lo:cfg.o_hi] = np.ascontiguousarray(
            _wrap16(ilo, MLO * P)).view(np.int32)
        eidx[B, :, cfg.o_hi:cfg.o_ad] = np.ascontiguousarray(
            _wrap16(ihi, MHI * P)).view(np.int32)
        eidx[B, :, cfg.o_ad:cfg.o_dr] = np.ascontiguousarray(
            _wrap16(iad.astype(np.int16), MB * P)).view(np.int32)
        eidx[B, :, cfg.o_dr:cfg.K] = drel.view(np.int32)

    return [np.ascontiguousarray(eidx[c * BPC:(c + 1) * BPC])
            for c in range(cfg.ncores)]


def compute_m(n, edge_index):
    src = np.asarray(edge_index[0]).astype(np.int64)
    dst = np.asarray(edge_index[1]).astype(np.int64)
    loop = np.arange(n, dtype=np.int64)
    src = np.concatenate([src, loop])
    dst = np.concatenate([dst, loop])
    blk = dst >> 7
    nblk = -(-n // P)
    lo = src < LO
    clo = np.bincount(blk[lo], minlength=nblk)
    chi = np.bincount(blk[~lo], minlength=nblk)
    return int(-(-clo.max() // P)), int(-(-chi.max() // P))


def make_in_maps(cfg, x, W1, att_src1, att_dst1, bias1, W2, att_src2,
                 att_dst2, bias2, edge_index):
    H1, CH1, HC1, C2 = cfg.h1, cfg.ch1, cfg.hc1, cfg.c2
    x = np.asarray(x, dtype=np.float32)
    xpad = np.zeros((cfg.npad, cfg.c_in), dtype=np.float32)
    xpad[: cfg.n] = x
    xt = np.ascontiguousarray(xpad.T).astype(np.float16)

    W1 = np.asarray(W1, np.float32)
    W2 = np.asarray(W2, np.float32)
    as1 = np.asarray(att_src1, np.float32)
    ad1 = np.asarray(att_dst1, np.float32)
    as2 = np.asarray(att_src2, np.float32).reshape(-1)
    ad2 = np.asarray(att_dst2, np.float32).reshape(-1)

    A1s = np.zeros((HC1, H1), dtype=np.float32)
    A1d = np.zeros((HC1, H1), dtype=np.float32)
    hh = np.repeat(np.arange(H1), CH1)
    A1s[np.arange(HC1), hh] = as1.reshape(-1)
    A1d[np.arange(HC1), hh] = ad1.reshape(-1)
    w1aug = np.concatenate([W1, W1 @ A1s, W1 @ A1d], axis=1).astype(np.float16)
    w2aug = np.concatenate([W2, (W2 @ as2)[:, None], (W2 @ ad2)[:, None]],
                           axis=1).astype(np.float16)

    colsum = W2.sum(axis=0)
    c0 = float(colsum @ (as2 + ad2))
    shifts = np.zeros((P, 2), dtype=np.float32)
    shifts[:, 0] = -2.0   # exp bias (cancels in softmax; keeps fp16 safe)
    shifts[:, 1] = -c0    # undo eluplus fold's logit shift (pre-leaky)

    b1b = np.tile(np.asarray(bias1, np.float32).reshape(1, -1),
                  (P, 1)).astype(np.float16)
    b2b = np.tile((np.asarray(bias2, np.float32).reshape(-1) - colsum
                   ).reshape(1, -1), (P, 1)).astype(np.float32)
    iota = np.tile(np.arange(P, dtype=np.float16), (P, 1))
    ident = np.eye(P, dtype=np.float16)

    per_core = host_prep(cfg, edge_index)
    G0 = cfg.nloc // (8 * P)
    in_maps = []
    for c in range(cfg.ncores):
        m = {"xt": xt, "w1aug": w1aug, "w2aug": w2aug, "b1b": b1b,
             "b2b": b2b, "iota": iota, "ident": ident, "shifts": shifts}
        base = c * cfg.bpc * P
        cols = np.minimum(base + np.arange(G0) * 8 * P,
                          cfg.npad - 8 * P).astype(np.int64)
        xoff = (np.arange(P)[:, None] * cfg.npad +
                cols[None, :]).astype(np.int32)
        m["xoff"] = xoff
        m["eidx"] = per_core[c]
        in_maps.append(m)
    return in_maps


_prog_cache = {}
_last_results = None


def kernel(x, edge_index, edge_weight, W1, att_src1, att_dst1, bias1,
           W2, att_src2, att_dst2, bias2):
    global _last_results
    n = x.shape[0]
    # edge_weight is unused (GATConv with edge_dim=None ignores it)
    mlo, mhi = compute_m(n, edge_index)
    mlo, mhi = max(mlo, 13), max(mhi, 8)

    cfg = Cfg(n, mlo, mhi, c_in=x.shape[1], h1=8, ch1=32, c2=64)
    key = (cfg.n, cfg.c_in, cfg.mlo, cfg.mhi)
    if key not in _prog_cache:
        _prog_cache[key] = build_program(cfg)
    nc = _prog_cache[key]

    in_maps = make_in_maps(cfg, x, W1, att_src1, att_dst1, bias1, W2,
                           att_src2, att_dst2, bias2, edge_index)
    res = run_bass_kernel_spmd(nc, in_maps, list(range(cfg.ncores)))
    _last_results = res
    outs = [res.results[c]["out"] for c in range(cfg.ncores)]
    full = np.concatenate(outs, axis=0)[: cfg.n]
    return np.ascontiguousarray(full)


# revision 19
# speedup vs baseline: 1167.5915x; 1.0963x over previous
"""GAT (2-layer, PyG-style) Trainium2 Bass kernel, 8-core SPMD. v3.

Strategy (dst-sharded edge aggregation, fp16 edge path):
- Host sorts edges by dst into 128-node blocks; within a block edges are
  ordered [src<32768 | src>=32768], each group padded to a multiple of 128
  (MLO/MHI chunks of 128 edge slots, global maxima). Core c owns blocks
  [c*BPC, (c+1)*BPC) and produces output rows for its own nodes only.
- Node tables live in DRAM with rows padded to 256B multiples so that
  InstDMAGatherAnt (int16 indices, one descriptor per edge, ~1us fixed cost
  per call) batches a whole block's gathers into 2 calls (low/high rows).
- a_dst is gathered from small per-core local tables whose indices fit
  int16: t1loc (written by phase 0) for layer 1, cc_in itself for layer 2.
- Edge softmax + scatter-add are expressed via one-hot S matrices + PE
  matmuls (fp16 in, fp32 PSUM accumulate). Logits are exp-shifted by -2
  (cancels exactly in softmax; keeps fp16 exp finite).
- ELU's "-1" is dropped (eluplus = relu(x)+exp(min(x,0))) and corrected at
  the end: out -= colsum(W2) (exact since softmax weights sum to 1); the
  induced constant layer-2 logit shift c0 is subtracted pre-leaky_relu.
- One AllGather of the 128-col fp16 layer-2 table is the only collective.
"""

import numpy as np

import concourse.bacc as bacc
import concourse.bass as bass
import concourse.mybir as mybir
import concourse.tile as tile
from concourse import library_config
from concourse.bass_utils import run_bass_kernel_spmd
from bass_rust import add_dep_helper


def _dep(a, b, reason):
    ia = a.ins if hasattr(a, "ins") else a
    ib = b.ins if hasattr(b, "ins") else b
    add_dep_helper(ia, ib, reason=reason)


P = 128
NCORES = 8
EPS = 1e-16
NEG_SLOPE = 0.2
LO = 32768                  # int16 index limit -> low/high table split
F32 = mybir.dt.float32
F16 = mybir.dt.float16
I32 = mybir.dt.int32
I16 = mybir.dt.int16
AF = mybir.ActivationFunctionType
ALU = mybir.AluOpType


class Cfg:
    def __init__(self, n_nodes, mlo, mhi, c_in=128, h1=8, ch1=32, c2=64,
                 ncores=NCORES):
        self.n = n_nodes
        self.c_in = c_in
        self.h1 = h1
        self.ch1 = ch1
        self.hc1 = h1 * ch1          # 256
        self.c2 = c2
        self.ncores = ncores
        self.bpc = -(-n_nodes // (P * ncores))      # 49
        self.npad = ncores * self.bpc * P
        self.nblk = ncores * self.bpc
        self.mlo = mlo
        self.mhi = mhi
        self.mb = mlo + mhi
        self.t1w = 384               # [h 256 | asrc 8 | adst 8 | pad]
        self.t2w = 128               # [h2 64 | asrc2 1 | adst2 1 | pad]
        self.nloc = ((self.bpc + 7) // 8) * 8 * P   # t1loc rows
        # eidx blob layout per block, in i32 columns:
        self.o_lo = 0                          # srclo idx16: mlo*4 i32 cols
        self.o_hi = self.o_lo + mlo * 4        # srchi idx16: mhi*4
        self.o_ad = self.o_hi + mhi * 4        # adst idx16:  mb*4
        self.o_dr = self.o_ad + self.mb * 4    # drel f32:    mb
        self.K = self.o_dr + self.mb



def _gpieces(nc, dep_fn, out_tile, cbase, W, in_ap, idx16, nchunks, dep, why):
    """Emit dma_gathers in <=512-index pieces (4 chunks of 128)."""
    for k0 in range(0, nchunks, 8):
        nk = min(8, nchunks - k0)
        g = nc.gpsimd.dma_gather(
            out_ap=out_tile[:, (cbase + k0) * W:(cbase + k0 + nk) * W]
                .rearrange("p (m w) -> p m w", m=nk),
            in_ap=in_ap, idxs_ap=idx16[:, k0 * 8:(k0 + nk) * 8],
            num_idxs=nk * 128, num_idxs_reg=nk * 128, elem_size=W)
        dep_fn(g, dep, why)


def build_program(cfg):
    nc = bacc.Bacc(None, num_devices=cfg.ncores)
    HC1, H1, CH1, C2 = cfg.hc1, cfg.h1, cfg.ch1, cfg.c2
    T1W, T2W, BPC = cfg.t1w, cfg.t2w, cfg.bpc
    MLO, MHI, MB = cfg.mlo, cfg.mhi, cfg.mb
    NBLK, NPAD, NLOC = cfg.nblk, cfg.npad, cfg.nloc
    G1 = 8
    assert NBLK % G1 == 0
    G0 = NLOC // (G1 * P)

    # ---- I/O ----
    xt = nc.dram_tensor("xt", [cfg.c_in, NPAD], F16, kind="ExternalInput")
    w1aug = nc.dram_tensor("w1aug", [cfg.c_in, 272], F16, kind="ExternalInput")
    w2aug = nc.dram_tensor("w2aug", [HC1, 66], F16, kind="ExternalInput")
    b1b = nc.dram_tensor("b1b", [P, HC1], F16, kind="ExternalInput")
    b2b = nc.dram_tensor("b2b", [P, C2], F32, kind="ExternalInput")
    iota = nc.dram_tensor("iota", [P, P], F16, kind="ExternalInput")
    ident = nc.dram_tensor("ident", [P, P], F16, kind="ExternalInput")
    shifts = nc.dram_tensor("shifts", [P, 2], F32, kind="ExternalInput")
    xoff = nc.dram_tensor("xoff", [P, G0], I32, kind="ExternalInput")
    eidx = nc.dram_tensor("eidx", [BPC, P, cfg.K], I32, kind="ExternalInput")
    out = nc.dram_tensor("out", [BPC * P, C2], F32, kind="ExternalOutput")

    # ---- internal DRAM ----
    t1 = nc.dram_tensor("t1", [NPAD, T1W], F16)
    t1loc = nc.dram_tensor("t1loc", [NLOC, T2W], F16)
    cc_in = nc.dram_tensor("cc_in", [BPC * P, T2W], F16)
    t2 = nc.dram_tensor("t2", [NPAD, T2W], F16, addr_space="Shared")

    groups = [list(range(cfg.ncores))]

    with tile.TileContext(nc) as tc:
        with (
            tc.tile_pool(name="const", bufs=1) as cpool,
            tc.tile_pool(name="p1", bufs=2) as p1pool,
            tc.tile_pool(name="blk", bufs=2) as bpool,
            tc.tile_pool(name="s", bufs=4) as spool,
            tc.tile_pool(name="fin", bufs=2) as opool,
            tc.tile_pool(name="ps", bufs=2, space="PSUM") as ps,
        ):
            # ---------------- constants ----------------
            iota_s = cpool.tile([P, P], F16)
            nc.sync.dma_start(out=iota_s[:], in_=iota[:])
            ident_s = cpool.tile([P, P], F16)
            nc.sync.dma_start(out=ident_s[:], in_=ident[:])
            w1aug_s = cpool.tile([P, 272], F16)
            nc.sync.dma_start(out=w1aug_s[:], in_=w1aug[:])
            w2aug_s = []
            for j in range(HC1 // P):
                wg = cpool.tile([P, 66], F16, tag=f"w2aug{j}")
                nc.sync.dma_start(out=wg[:], in_=w2aug[j * P:(j + 1) * P, :])
                w2aug_s.append(wg)
            b1b_s = cpool.tile([P, HC1], F16)
            nc.sync.dma_start(out=b1b_s[:], in_=b1b[:])
            b2b_s = cpool.tile([P, C2], F32)
            nc.sync.dma_start(out=b2b_s[:], in_=b2b[:])
            shifts_s = cpool.tile([P, 2], F32)
            nc.sync.dma_start(out=shifts_s[:], in_=shifts[:])
            xoff_s = cpool.tile([P, G0], I32)
            nc.sync.dma_start(out=xoff_s[:], in_=xoff[:])

            # ------------- phase 0: local a_dst table (own blocks) -------
            loc_writes = []
            for g in range(G0):
                xg0 = p1pool.tile([P, G1 * P], F16, tag="xg0")
                nc.gpsimd.indirect_dma_start(
                    out=xg0[:], out_offset=None, in_=xt[:],
                    in_offset=bass.IndirectOffsetOnAxis(
                        ap=xoff_s[:, g:g + 1], axis=1))
                rows0 = p1pool.tile([P, G1 * H1], F16, tag="rows0")
                for j in range(G1):
                    pl = ps.tile([P, H1], F32, space="PSUM", tag="den")
                    nc.tensor.matmul(out=pl[:], lhsT=xg0[:, j * P:(j + 1) * P],
                                     rhs=w1aug_s[:, 264:272],
                                     start=True, stop=True)
                    nc.vector.tensor_scalar_add(
                        out=rows0[:, j * H1:(j + 1) * H1], in0=pl[:],
                        scalar1=0.0)
                loc_writes.append(nc.sync.dma_start(
                    out=t1loc[g * G1 * P:(g + 1) * G1 * P, 0:H1].rearrange(
                        "(j p) c -> p j c", j=G1),
                    in_=rows0[:].rearrange("p (j c) -> p j c", j=G1)))
            jloc_t = cpool.tile([1, 1], F32, tag="jloc")
            jloc = nc.gpsimd.memset(jloc_t[:], 0.0)
            for w in loc_writes:
                _dep(jloc, w, "adst gathers wait for local table")

            # ------------- phase 1: full node table (replicated) ---------
            t1_writes = []
            for grp in range(NBLK // G1):
                B0 = grp * G1
                xg = p1pool.tile([P, G1 * P], F16, tag="xg")
                nc.sync.dma_start(out=xg[:], in_=xt[:, B0 * P:(B0 + G1) * P])
                rows = p1pool.tile([P, G1 * 272], F16, tag="rows")
                for j in range(G1):
                    ph1 = ps.tile([P, 272], F32, space="PSUM", tag="acc")
                    nc.tensor.matmul(out=ph1[:], lhsT=xg[:, j * P:(j + 1) * P],
                                     rhs=w1aug_s[:], start=True, stop=True)
                    dst = rows[:, j * 272:(j + 1) * 272]
                    if j % 2 == 0:
                        nc.scalar.copy(out=dst, in_=ph1[:])
                    else:
                        nc.vector.tensor_scalar_add(out=dst, in0=ph1[:],
                                                    scalar1=0.0)
                t1_writes.append(nc.sync.dma_start(
                    out=t1[B0 * P:(B0 + G1) * P, 0:272].rearrange(
                        "(j p) c -> p j c", j=G1),
                    in_=rows[:].rearrange("p (j c) -> p j c", j=G1)))

            j1tile = cpool.tile([1, 1], F32, tag="j1")
            j1 = nc.gpsimd.memset(j1tile[:], 0.0)
            for w in t1_writes:
                _dep(j1, w, "layer1 gathers wait for full node table")

            # gpsimd ext-isa library containing InstDMAGatherAnt
            nc.gpsimd.load_library(library_config.mlp)

            # ------------- phase 2: layer-1 edge aggregation -------------
            cc_writes = []
            for b in range(BPC):
                r0 = b * P
                tlo = bpool.tile([P, MLO * 4], I32, tag="tlo")
                nc.sync.dma_start(out=tlo[:], in_=eidx[b, :, cfg.o_lo:cfg.o_hi])
                thi = bpool.tile([P, MHI * 4], I32, tag="thi")
                nc.sync.dma_start(out=thi[:], in_=eidx[b, :, cfg.o_hi:cfg.o_ad])
                tad = bpool.tile([P, MB * 4], I32, tag="tad")
                nc.sync.dma_start(out=tad[:], in_=eidx[b, :, cfg.o_ad:cfg.o_dr])
                tdr = bpool.tile([P, MB], I32, tag="tdr")
                nc.sync.dma_start(out=tdr[:], in_=eidx[b, :, cfg.o_dr:cfg.K])
                ilo = tlo[:].bitcast(I16)
                ihi = thi[:].bitcast(I16)
                iad = tad[:].bitcast(I16)
                drel = tdr[:].bitcast(F32)

                gath = bpool.tile([P, MB * T1W], F16, tag="gath")
                _gpieces(nc, _dep, gath, 0, T1W, t1[0:LO, :], ilo, MLO,
                         j1, "lo gather after table1")
                _gpieces(nc, _dep, gath, MLO, T1W, t1[LO:NPAD, :], ihi, MHI,
                         j1, "hi gather after table1")
                adstg = bpool.tile([P, MB * T2W], F16, tag="adstg")
                _gpieces(nc, _dep, adstg, 0, T2W, t1loc[:], iad, MB,
                         jloc, "adst gather after local table")

                gv = gath[:].rearrange("p (m w) -> p m w", m=MB)
                av = bpool.tile([P, MB * H1], F16, tag="av")
                nc.vector.tensor_tensor(
                    out=av[:].rearrange("p (m h) -> p m h", m=MB),
                    in0=gv[:, :, 256:264],
                    in1=adstg[:].rearrange("p (m w) -> p m w",
                                           m=MB)[:, :, 0:H1],
                    op=ALU.add)
                lk = bpool.tile([P, MB * H1], F16, tag="lk")
                nc.vector.scalar_tensor_tensor(
                    out=lk[:], in0=av[:], scalar=NEG_SLOPE, in1=av[:],
                    op0=ALU.mult, op1=ALU.max)
                efull = bpool.tile([P, MB * HC1], F16, tag="efull")
                nc.scalar.activation(
                    out=efull[:],
                    in_=lk[:].rearrange("p (m h) -> p m h", m=MB)
                          .to_broadcast([P, MB, H1, CH1]),
                    func=AF.Exp, bias=shifts_s[:, 0:1])
                wm = bpool.tile([P, MB * HC1], F16, tag="wm")
                nc.vector.tensor_tensor(
                    out=wm[:].rearrange("p (m c) -> p m c", m=MB),
                    in0=gv[:, :, 0:HC1],
                    in1=efull[:].rearrange("p (m c) -> p m c", m=MB),
                    op=ALU.mult)

                pacc = ps.tile([P, 272], F32, space="PSUM", tag="acc")
                pmsg = pacc[:, 0:HC1]
                pdent = ps.tile([P, H1], F32, space="PSUM", tag="den")
                pden = pdent[:]
                e4 = efull[:].rearrange("p (m h c) -> p m h c", m=MB, h=H1)
                for g in range(MB):
                    S = spool.tile([P, P], F16, tag="S")
                    nc.vector.tensor_scalar(
                        out=S[:], in0=iota_s[:], scalar1=drel[:, g:g + 1],
                        scalar2=None, op0=ALU.is_equal)
                    nc.tensor.matmul(out=pmsg, lhsT=S[:],
                                     rhs=wm[:, g * HC1:(g + 1) * HC1],
                                     start=(g == 0), stop=(g == MB - 1))
                    nc.tensor.matmul(out=pden, lhsT=S[:],
                                     rhs=e4[:, g, :, 0],
                                     start=(g == 0), stop=(g == MB - 1))

                den = opool.tile([P, H1], F32, tag="den")
                nc.vector.tensor_scalar_add(out=den[:], in0=pden,
                                            scalar1=EPS)
                rec = opool.tile([P, H1], F32, tag="rec")
                nc.vector.reciprocal(out=rec[:], in_=den[:])
                o1b = opool.tile([P, HC1], F16, tag="o1b")
                for h in range(H1):
                    sl = slice(h * CH1, (h + 1) * CH1)
                    nc.vector.scalar_tensor_tensor(
                        out=o1b[:, sl], in0=pacc[:, sl],
                        scalar=rec[:, h:h + 1], in1=b1b_s[:, sl],
                        op0=ALU.mult, op1=ALU.add)
                xn = opool.tile([P, HC1], F16, tag="xn")
                nc.gpsimd.tensor_scalar_min(out=xn[:], in0=o1b[:], scalar1=0.0)
                en = opool.tile([P, HC1], F16, tag="en")
                nc.scalar.activation(out=en[:], in_=xn[:], func=AF.Exp)
                helu = opool.tile([P, HC1], F16, tag="helu")
                nc.vector.scalar_tensor_tensor(
                    out=helu[:], in0=o1b[:], scalar=0.0, in1=en[:],
                    op0=ALU.max, op1=ALU.add)

                ph2 = ps.tile([P, 66], F32, space="PSUM", tag="ph2")
                for j in range(HC1 // P):
                    pT = ps.tile([P, P], F16, space="PSUM", tag="pT")
                    nc.tensor.transpose(out=pT[:],
                                        in_=helu[:, j * P:(j + 1) * P],
                                        identity=ident_s[:])
                    hT = opool.tile([P, P], F16, tag="hT")
                    nc.vector.tensor_scalar_add(out=hT[:], in0=pT[:],
                                                scalar1=0.0)
                    nc.tensor.matmul(out=ph2[:], lhsT=hT[:], rhs=w2aug_s[j][:],
                                     start=(j == 0), stop=(j == HC1 // P - 1))
                h2row = opool.tile([P, T2W], F16, tag="h2row")
                nc.vector.tensor_scalar_add(out=h2row[:, 0:66], in0=ph2[:],
                                            scalar1=0.0)
                nc.vector.memset(h2row[:, 66:T2W], 0.0)
                cc_writes.append(nc.sync.dma_start(
                    out=cc_in[r0:r0 + P, :], in_=h2row[:]))

            # ------------- phase 3: share layer-2 node table -------------
            nc.gpsimd.load_library(library_config.standard)
            cc = nc.gpsimd.collective_compute(
                "AllGather", ALU.bypass, replica_groups=groups,
                ins=[cc_in[:]], outs=[t2[:]])
            for w in cc_writes:
                _dep(cc, w, "allgather after cc writes")
            j2tile = cpool.tile([1, 1], F32, tag="j2")
            j2 = nc.gpsimd.memset(j2tile[:], 0.0)
            _dep(j2, cc, "layer2 gathers after allgather")
            nc.gpsimd.load_library(library_config.mlp)

            # ------------- phase 4: layer-2 edge aggregation -------------
            for b in range(BPC):
                r0 = b * P
                tlo = bpool.tile([P, MLO * 4], I32, tag="tlo2")
                nc.sync.dma_start(out=tlo[:], in_=eidx[b, :, cfg.o_lo:cfg.o_hi])
                thi = bpool.tile([P, MHI * 4], I32, tag="thi2")
                nc.sync.dma_start(out=thi[:], in_=eidx[b, :, cfg.o_hi:cfg.o_ad])
                tad = bpool.tile([P, MB * 4], I32, tag="tad2")
                nc.sync.dma_start(out=tad[:], in_=eidx[b, :, cfg.o_ad:cfg.o_dr])
                tdr = bpool.tile([P, MB], I32, tag="tdr2")
                nc.sync.dma_start(out=tdr[:], in_=eidx[b, :, cfg.o_dr:cfg.K])
                ilo = tlo[:].bitcast(I16)
                ihi = thi[:].bitcast(I16)
                iad = tad[:].bitcast(I16)
                drel = tdr[:].bitcast(F32)

                gath2 = bpool.tile([P, MB * T2W], F16, tag="gath2")
                _gpieces(nc, _dep, gath2, 0, T2W, t2[0:LO, :], ilo, MLO,
                         j2, "lo gather after table2")
                _gpieces(nc, _dep, gath2, MLO, T2W, t2[LO:NPAD, :], ihi, MHI,
                         j2, "hi gather after table2")
                adst2 = bpool.tile([P, MB * T2W], F16, tag="adst2")
                _gpieces(nc, _dep, adst2, 0, T2W, cc_in[:], iad, MB,
                         j2, "adst2 gather after cc writes")

                qv = gath2[:].rearrange("p (m w) -> p m w", m=MB)
                av2 = bpool.tile([P, MB], F16, tag="av2")
                nc.vector.scalar_tensor_tensor(
                    out=av2[:].rearrange("p (m o) -> p m o", m=MB),
                    in0=qv[:, :, 64:65], scalar=shifts_s[:, 1:2],
                    in1=adst2[:].rearrange("p (m w) -> p m w",
                                           m=MB)[:, :, 65:66],
                    op0=ALU.add, op1=ALU.add)
                lk2 = bpool.tile([P, MB], F16, tag="lk2")
                nc.vector.scalar_tensor_tensor(
                    out=lk2[:], in0=av2[:], scalar=NEG_SLOPE, in1=av2[:],
                    op0=ALU.mult, op1=ALU.max)
                e2full = bpool.tile([P, MB * C2], F16, tag="e2full")
                nc.scalar.activation(
                    out=e2full[:],
                    in_=lk2[:].rearrange("p (m o) -> p m o", m=MB)
                           .to_broadcast([P, MB, 1, C2]),
                    func=AF.Exp, bias=shifts_s[:, 0:1])
                wm2 = bpool.tile([P, MB * C2], F16, tag="wm2")
                nc.vector.tensor_tensor(
                    out=wm2[:].rearrange("p (m c) -> p m c", m=MB),
                    in0=qv[:, :, 0:C2],
                    in1=e2full[:].rearrange("p (m c) -> p m c", m=MB),
                    op=ALU.mult)

                pacc2 = ps.tile([P, 272], F32, space="PSUM", tag="acc")
                pmsg2 = pacc2[:, 0:C2]
                pdent2 = ps.tile([P, H1], F32, space="PSUM", tag="den")
                pden2 = pdent2[:, 0:1]
                for g in range(MB):
                    S = spool.tile([P, P], F16, tag="S")
                    nc.vector.tensor_scalar(
                        out=S[:], in0=iota_s[:], scalar1=drel[:, g:g + 1],
                        scalar2=None, op0=ALU.is_equal)
                    nc.tensor.matmul(out=pmsg2, lhsT=S[:],
                                     rhs=wm2[:, g * C2:(g + 1) * C2],
                                     start=(g == 0), stop=(g == MB - 1))
                    nc.tensor.matmul(out=pden2, lhsT=S[:],
                                     rhs=e2full[:, g * C2:g * C2 + 1],
                                     start=(g == 0), stop=(g == MB - 1))

                den2 = opool.tile([P, 1], F32, tag="den2")
                nc.vector.tensor_scalar_add(out=den2[:], in0=pden2,
                                            scalar1=EPS)
                rec2 = opool.tile([P, 1], F32, tag="rec2")
                nc.vector.reciprocal(out=rec2[:], in_=den2[:])
                o2 = opool.tile([P, C2], F32, tag="o2")
                nc.vector.scalar_tensor_tensor(
                    out=o2[:], in0=pmsg2, scalar=rec2[:, 0:1],
                    in1=b2b_s[:], op0=ALU.mult, op1=ALU.add)
                nc.sync.dma_start(out=out[r0:r0 + P, :], in_=o2[:])

    nc.compile()
    return nc


def _wrap16(idx, nid):
    """Pack an int16 index list (len nid) into a [128, nid//16] tile:
    element k at (k%16, k//16), replicated to partitions 16..127."""
    a = np.asarray(idx, np.int16).reshape(nid // 16, 16).T  # [16, nid//16]
    return np.tile(a, (8, 1))


def host_prep(cfg, edge_index):
    n = cfg.n
    src = np.asarray(edge_index[0]).astype(np.int64)
    dst = np.asarray(edge_index[1]).astype(np.int64)
    loop = np.arange(n, dtype=np.int64)
    src = np.concatenate([src, loop])
    dst = np.concatenate([dst, loop])

    order = np.argsort(dst, kind="stable")
    ss = src[order]
    ds = dst[order]
    blk = ds >> 7

    MLO, MHI, MB = cfg.mlo, cfg.mhi, cfg.mb
    NBLK, BPC = cfg.nblk, cfg.bpc
    eidx = np.zeros((NBLK, P, cfg.K), dtype=np.int32)

    starts = np.zeros(NBLK + 1, dtype=np.int64)
    np.cumsum(np.bincount(blk, minlength=NBLK), out=starts[1:])

    for B in range(NBLK):
        s_b = ss[starts[B]:starts[B + 1]]
        d_b = ds[starts[B]:starts[B + 1]]
        lo_m = s_b < LO
        s_lo, d_lo = s_b[lo_m], d_b[lo_m]
        s_hi, d_hi = s_b[~lo_m], d_b[~lo_m]
        nlo, nhi = len(s_lo), len(s_hi)
        assert -(-nlo // P) <= MLO and -(-nhi // P) <= MHI, (B, nlo, nhi)
        cbase = (B // BPC) * BPC * P

        ilo = np.zeros(MLO * P, np.int16)
        ilo[:nlo] = s_lo.astype(np.int16)
        ihi = np.zeros(MHI * P, np.int16)
        ihi[:nhi] = (s_hi - LO).astype(np.int16)
        kidx = np.concatenate([np.arange(nlo), MLO * P + np.arange(nhi)])
        d_all = np.concatenate([d_lo, d_hi])
        iad = np.zeros(MB * P, np.int64)
        iad[kidx] = d_all - cbase
        drel = np.full((P, MB), -1.0, dtype=np.float32)
        drel[kidx % P, kidx // P] = (d_all - (B << 7)).astype(np.float32)

        eidx[B, :, cfg.o_lo:cfg.o_hi] = np.ascontiguousarray(
            _wrap16(ilo, MLO * P)).view(np.int32)
        eidx[B, :, cfg.o_hi:cfg.o_ad] = np.ascontiguousarray(
            _wrap16(ihi, MHI * P)).view(np.int32)
        eidx[B, :, cfg.o_ad:cfg.o_dr] = np.ascontiguousarray(
            _wrap16(iad.astype(np.int16), MB * P)).view(np.int32)
        eidx[B, :, cfg.o_dr:cfg.K] = drel.view(np.int32)

    return [np.ascontiguousarray(eidx[c * BPC:(c + 1) * BPC])
            for c in range(cfg.ncores)]


def compute_m(n, edge_index):
    src = np.asarray(edge_index[0]).astype(np.int64)
    dst = np.asarray(edge_index[1]).astype(np.int64)
    loop = np.arange(n, dtype=np.int64)
    src = np.concatenate([src, loop])
    dst = np.concatenate([dst, loop])
    blk = dst >> 7
    nblk = -(-n // P)
    lo = src < LO
    clo = np.bincount(blk[lo], minlength=nblk)
    chi = np.bincount(blk[~lo], minlength=nblk)
    return int(-(-clo.max() // P)), int(-(-chi.max() // P))


def make_in_maps(cfg, x, W1, att_src1, att_dst1, bias1, W2, att_src2,
                 att_dst2, bias2, edge_index):
    H1, CH1, HC1, C2 = cfg.h1, cfg.ch1, cfg.hc1, cfg.c2
    x = np.asarray(x, dtype=np.float32)
    xpad = np.zeros((cfg.npad, cfg.c_in), dtype=np.float32)
    xpad[: cfg.n] = x
    xt = np.ascontiguousarray(xpad.T).astype(np.float16)

    W1 = np.asarray(W1, np.float32)
    W2 = np.asarray(W2, np.float32)
    as1 = np.asarray(att_src1, np.float32)
    ad1 = np.asarray(att_dst1, np.float32)
    as2 = np.asarray(att_src2, np.float32).reshape(-1)
    ad2 = np.asarray(att_dst2, np.float32).reshape(-1)

    A1s = np.zeros((HC1, H1), dtype=np.float32)
    A1d = np.zeros((HC1, H1), dtype=np.float32)
    hh = np.repeat(np.arange(H1), CH1)
    A1s[np.arange(HC1), hh] = as1.reshape(-1)
    A1d[np.arange(HC1), hh] = ad1.reshape(-1)
    w1aug = np.concatenate([W1, W1 @ A1s, W1 @ A1d], axis=1).astype(np.float16)
    w2aug = np.concatenate([W2, (W2 @ as2)[:, None], (W2 @ ad2)[:, None]],
                           axis=1).astype(np.float16)

    colsum = W2.sum(axis=0)
    c0 = float(colsum @ (as2 + ad2))
    shifts = np.zeros((P, 2), dtype=np.float32)
    shifts[:, 0] = -2.0   # exp bias (cancels in softmax; keeps fp16 safe)
    shifts[:, 1] = -c0    # undo eluplus fold's logit shift (pre-leaky)

    b1b = np.tile(np.asarray(bias1, np.float32).reshape(1, -1),
                  (P, 1)).astype(np.float16)
    b2b = np.tile((np.asarray(bias2, np.float32).reshape(-1) - colsum
                   ).reshape(1, -1), (P, 1)).astype(np.float32)
    iota = np.tile(np.arange(P, dtype=np.float16), (P, 1))
    ident = np.eye(P, dtype=np.float16)

    per_core = host_prep(cfg, edge_index)
    G0 = cfg.nloc // (8 * P)
    in_maps = []
    for c in range(cfg.ncores):
        m = {"xt": xt, "w1aug": w1aug, "w2aug": w2aug, "b1b": b1b,
             "b2b": b2b, "iota": iota, "ident": ident, "shifts": shifts}
        base = c * cfg.bpc * P
        cols = np.minimum(base + np.arange(G0) * 8 * P,
                          cfg.npad - 8 * P).astype(np.int64)
        xoff = (np.arange(P)[:, None] * cfg.npad +
                cols[None, :]).astype(np.int32)
        m["xoff"] = xoff
        m["eidx"] = per_core[c]
        in_maps.append(m)
    return in_maps


_prog_cache = {}
_last_results = None


def kernel(x, edge_index, edge_weight, W1, att_src1, att_dst1, bias1,
           W2, att_src2, att_dst2, bias2):
    global _last_results
    n = x.shape[0]
    # edge_weight is unused (GATConv with edge_dim=None ignores it)
    mlo, mhi = compute_m(n, edge_index)
    mlo, mhi = max(mlo, 13), max(mhi, 8)

    cfg = Cfg(n, mlo, mhi, c_in=x.shape[1], h1=8, ch1=32, c2=64)
    key = (cfg.n, cfg.c_in, cfg.mlo, cfg.mhi)
    if key not in _prog_cache:
        _prog_cache[key] = build_program(cfg)
    nc = _prog_cache[key]

    in_maps = make_in_maps(cfg, x, W1, att_src1, att_dst1, bias1, W2,
                           att_src2, att_dst2, bias2, edge_index)
    res = run_bass_kernel_spmd(nc, in_maps, list(range(cfg.ncores)))
    _last_results = res
    outs = [res.results[c]["out"] for c in range(cfg.ncores)]
    full = np.concatenate(outs, axis=0)[: cfg.n]
    return np.ascontiguousarray(full)


# revision 20
# speedup vs baseline: 1719.4820x; 1.4727x over previous
"""GAT (2-layer, PyG-style) Trainium2 Bass kernel, 8-core SPMD. v3.

Strategy (dst-sharded edge aggregation, fp16 edge path):
- Host sorts edges by dst into 128-node blocks; within a block edges are
  ordered [src<32768 | src>=32768], each group padded to a multiple of 128
  (MLO/MHI chunks of 128 edge slots, global maxima). Core c owns blocks
  [c*BPC, (c+1)*BPC) and produces output rows for its own nodes only.
- Node tables live in DRAM with rows padded to 256B multiples so that
  InstDMAGatherAnt (int16 indices, one descriptor per edge, ~1us fixed cost
  per call) batches a whole block's gathers into 2 calls (low/high rows).
- a_dst is gathered from small per-core local tables whose indices fit
  int16: t1loc (written by phase 0) for layer 1, cc_in itself for layer 2.
- Edge softmax + scatter-add are expressed via one-hot S matrices + PE
  matmuls (fp16 in, fp32 PSUM accumulate). Logits are exp-shifted by -2
  (cancels exactly in softmax; keeps fp16 exp finite).
- ELU's "-1" is dropped (eluplus = relu(x)+exp(min(x,0))) and corrected at
  the end: out -= colsum(W2) (exact since softmax weights sum to 1); the
  induced constant layer-2 logit shift c0 is subtracted pre-leaky_relu.
- One AllGather of the 128-col fp16 layer-2 table is the only collective.
"""

import numpy as np

import concourse.bacc as bacc
import concourse.bass as bass
import concourse.mybir as mybir
import concourse.tile as tile
from concourse import library_config
from concourse.bass_utils import run_bass_kernel_spmd
from bass_rust import add_dep_helper


def _dep(a, b, reason):
    ia = a.ins if hasattr(a, "ins") else a
    ib = b.ins if hasattr(b, "ins") else b
    add_dep_helper(ia, ib, reason=reason)


P = 128
NCORES = 8
EPS = 1e-16
NEG_SLOPE = 0.2
LO = 32768                  # int16 index limit -> low/high table split
F32 = mybir.dt.float32
F16 = mybir.dt.float16
I32 = mybir.dt.int32
I16 = mybir.dt.int16
AF = mybir.ActivationFunctionType
ALU = mybir.AluOpType


class Cfg:
    def __init__(self, n_nodes, mlo, mhi, c_in=128, h1=8, ch1=32, c2=64,
                 ncores=NCORES):
        self.n = n_nodes
        self.c_in = c_in
        self.h1 = h1
        self.ch1 = ch1
        self.hc1 = h1 * ch1          # 256
        self.c2 = c2
        self.ncores = ncores
        self.bpc = -(-n_nodes // (P * ncores))      # 49
        self.npad = ncores * self.bpc * P
        self.nblk = ncores * self.bpc
        self.mlo = mlo
        self.mhi = mhi
        self.mb = mlo + mhi
        self.t1w = 384               # [h 256 | asrc 8 | adst 8 | pad]
        self.t2w = 128               # [h2 64 | asrc2 1 | adst2 1 | pad]
        self.nloc = ((self.bpc + 7) // 8) * 8 * P   # t1loc rows
        # eidx blob layout per block, in i32 columns:
        self.o_lo = 0                          # srclo idx16: mlo*4 i32 cols
        self.o_hi = self.o_lo + mlo * 4        # srchi idx16: mhi*4
        self.o_ad = self.o_hi + mhi * 4        # adst idx16:  mb*4
        self.o_dr = self.o_ad + self.mb * 4    # drel f32:    mb
        self.K = self.o_dr + self.mb



_GQ = [0]


def _gpieces(nc, dep_fn, out_tile, cbase, W, in_ap, idx16, nchunks, dep, why):
    """Emit dma_gathers in <=1024-index pieces, alternating SWDGE queues."""
    for k0 in range(0, nchunks, 8):
        nk = min(8, nchunks - k0)
        q = _GQ[0] % 2
        _GQ[0] += 1
        g = nc.gpsimd.dma_gather(
            out_ap=out_tile[:, (cbase + k0) * W:(cbase + k0 + nk) * W]
                .rearrange("p (m w) -> p m w", m=nk),
            in_ap=in_ap, idxs_ap=idx16[:, k0 * 8:(k0 + nk) * 8],
            num_idxs=nk * 128, num_idxs_reg=nk * 128, elem_size=W,
            queue_num=q)
        dep_fn(g, dep, why)


def build_program(cfg):
    nc = bacc.Bacc(None, num_devices=cfg.ncores, num_swdge_queues=2)
    HC1, H1, CH1, C2 = cfg.hc1, cfg.h1, cfg.ch1, cfg.c2
    T1W, T2W, BPC = cfg.t1w, cfg.t2w, cfg.bpc
    MLO, MHI, MB = cfg.mlo, cfg.mhi, cfg.mb
    NBLK, NPAD, NLOC = cfg.nblk, cfg.npad, cfg.nloc
    G1 = 8
    assert NBLK % G1 == 0
    G0 = NLOC // (G1 * P)

    # ---- I/O ----
    xt = nc.dram_tensor("xt", [cfg.c_in, NPAD], F16, kind="ExternalInput")
    w1aug = nc.dram_tensor("w1aug", [cfg.c_in, 272], F16, kind="ExternalInput")
    w2aug = nc.dram_tensor("w2aug", [HC1, 66], F16, kind="ExternalInput")
    b1b = nc.dram_tensor("b1b", [P, HC1], F16, kind="ExternalInput")
    b2b = nc.dram_tensor("b2b", [P, C2], F32, kind="ExternalInput")
    iota = nc.dram_tensor("iota", [P, P], F16, kind="ExternalInput")
    ident = nc.dram_tensor("ident", [P, P], F16, kind="ExternalInput")
    shifts = nc.dram_tensor("shifts", [P, 2], F32, kind="ExternalInput")
    xoff = nc.dram_tensor("xoff", [P, G0], I32, kind="ExternalInput")
    eidx = nc.dram_tensor("eidx", [BPC, P, cfg.K], I32, kind="ExternalInput")
    out = nc.dram_tensor("out", [BPC * P, C2], F32, kind="ExternalOutput")

    # ---- internal DRAM ----
    t1 = nc.dram_tensor("t1", [NPAD, T1W], F16)
    t1loc = nc.dram_tensor("t1loc", [NLOC, T2W], F16)
    cc_in = nc.dram_tensor("cc_in", [BPC * P, T2W], F16)
    t2 = nc.dram_tensor("t2", [NPAD, T2W], F16, addr_space="Shared")

    groups = [list(range(cfg.ncores))]

    with tile.TileContext(nc) as tc:
        with (
            tc.tile_pool(name="const", bufs=1) as cpool,
            tc.tile_pool(name="p1", bufs=2) as p1pool,
            tc.tile_pool(name="blk", bufs=2) as bpool,
            tc.tile_pool(name="s", bufs=4) as spool,
            tc.tile_pool(name="fin", bufs=2) as opool,
            tc.tile_pool(name="ps", bufs=2, space="PSUM") as ps,
        ):
            # ---------------- constants ----------------
            iota_s = cpool.tile([P, P], F16)
            nc.sync.dma_start(out=iota_s[:], in_=iota[:])
            ident_s = cpool.tile([P, P], F16)
            nc.sync.dma_start(out=ident_s[:], in_=ident[:])
            w1aug_s = cpool.tile([P, 272], F16)
            nc.sync.dma_start(out=w1aug_s[:], in_=w1aug[:])
            w2aug_s = []
            for j in range(HC1 // P):
                wg = cpool.tile([P, 66], F16, tag=f"w2aug{j}")
                nc.sync.dma_start(out=wg[:], in_=w2aug[j * P:(j + 1) * P, :])
                w2aug_s.append(wg)
            b1b_s = cpool.tile([P, HC1], F16)
            nc.sync.dma_start(out=b1b_s[:], in_=b1b[:])
            b2b_s = cpool.tile([P, C2], F32)
            nc.sync.dma_start(out=b2b_s[:], in_=b2b[:])
            shifts_s = cpool.tile([P, 2], F32)
            nc.sync.dma_start(out=shifts_s[:], in_=shifts[:])
            xoff_s = cpool.tile([P, G0], I32)
            nc.sync.dma_start(out=xoff_s[:], in_=xoff[:])

            # ------------- phase 0: local a_dst table (own blocks) -------
            loc_writes = []
            for g in range(G0):
                xg0 = p1pool.tile([P, G1 * P], F16, tag="xg0")
                nc.gpsimd.indirect_dma_start(
                    out=xg0[:], out_offset=None, in_=xt[:],
                    in_offset=bass.IndirectOffsetOnAxis(
                        ap=xoff_s[:, g:g + 1], axis=1))
                rows0 = p1pool.tile([P, G1 * H1], F16, tag="rows0")
                for j in range(G1):
                    pl = ps.tile([P, H1], F32, space="PSUM", tag="den")
                    nc.tensor.matmul(out=pl[:], lhsT=xg0[:, j * P:(j + 1) * P],
                                     rhs=w1aug_s[:, 264:272],
                                     start=True, stop=True)
                    nc.vector.tensor_scalar_add(
                        out=rows0[:, j * H1:(j + 1) * H1], in0=pl[:],
                        scalar1=0.0)
                loc_writes.append(nc.sync.dma_start(
                    out=t1loc[g * G1 * P:(g + 1) * G1 * P, 0:H1].rearrange(
                        "(j p) c -> p j c", j=G1),
                    in_=rows0[:].rearrange("p (j c) -> p j c", j=G1)))
            jloc_t = cpool.tile([1, 1], F32, tag="jloc")
            jloc = nc.gpsimd.memset(jloc_t[:], 0.0)
            for w in loc_writes:
                _dep(jloc, w, "adst gathers wait for local table")

            # ------------- phase 1: full node table (replicated) ---------
            t1_writes = []
            for grp in range(NBLK // G1):
                B0 = grp * G1
                xg = p1pool.tile([P, G1 * P], F16, tag="xg")
                nc.sync.dma_start(out=xg[:], in_=xt[:, B0 * P:(B0 + G1) * P])
                rows = p1pool.tile([P, G1 * 272], F16, tag="rows")
                for j in range(G1):
                    ph1 = ps.tile([P, 272], F32, space="PSUM", tag="acc")
                    nc.tensor.matmul(out=ph1[:], lhsT=xg[:, j * P:(j + 1) * P],
                                     rhs=w1aug_s[:], start=True, stop=True)
                    dst = rows[:, j * 272:(j + 1) * 272]
                    if j % 2 == 0:
                        nc.scalar.copy(out=dst, in_=ph1[:])
                    else:
                        nc.vector.tensor_scalar_add(out=dst, in0=ph1[:],
                                                    scalar1=0.0)
                t1_writes.append(nc.sync.dma_start(
                    out=t1[B0 * P:(B0 + G1) * P, 0:272].rearrange(
                        "(j p) c -> p j c", j=G1),
                    in_=rows[:].rearrange("p (j c) -> p j c", j=G1)))

            j1tile = cpool.tile([1, 1], F32, tag="j1")
            j1 = nc.gpsimd.memset(j1tile[:], 0.0)
            for w in t1_writes:
                _dep(j1, w, "layer1 gathers wait for full node table")

            # gpsimd ext-isa library containing InstDMAGatherAnt
            nc.gpsimd.load_library(library_config.mlp)

            # ------------- phase 2: layer-1 edge aggregation -------------
            cc_writes = []
            for b in range(BPC):
                r0 = b * P
                tlo = bpool.tile([P, MLO * 4], I32, tag="tlo")
                nc.sync.dma_start(out=tlo[:], in_=eidx[b, :, cfg.o_lo:cfg.o_hi])
                thi = bpool.tile([P, MHI * 4], I32, tag="thi")
                nc.sync.dma_start(out=thi[:], in_=eidx[b, :, cfg.o_hi:cfg.o_ad])
                tad = bpool.tile([P, MB * 4], I32, tag="tad")
                nc.sync.dma_start(out=tad[:], in_=eidx[b, :, cfg.o_ad:cfg.o_dr])
                tdr = bpool.tile([P, MB], I32, tag="tdr")
                nc.sync.dma_start(out=tdr[:], in_=eidx[b, :, cfg.o_dr:cfg.K])
                ilo = tlo[:].bitcast(I16)
                ihi = thi[:].bitcast(I16)
                iad = tad[:].bitcast(I16)
                drel = tdr[:].bitcast(F32)

                gath = bpool.tile([P, MB * T1W], F16, tag="gath")
                _gpieces(nc, _dep, gath, 0, T1W, t1[0:LO, :], ilo, MLO,
                         j1, "lo gather after table1")
                _gpieces(nc, _dep, gath, MLO, T1W, t1[LO:NPAD, :], ihi, MHI,
                         j1, "hi gather after table1")
                adstg = bpool.tile([P, MB * T2W], F16, tag="adstg")
                _gpieces(nc, _dep, adstg, 0, T2W, t1loc[:], iad, MB,
                         jloc, "adst gather after local table")

                gv = gath[:].rearrange("p (m w) -> p m w", m=MB)
                av = bpool.tile([P, MB * H1], F16, tag="av")
                nc.vector.tensor_tensor(
                    out=av[:].rearrange("p (m h) -> p m h", m=MB),
                    in0=gv[:, :, 256:264],
                    in1=adstg[:].rearrange("p (m w) -> p m w",
                                           m=MB)[:, :, 0:H1],
                    op=ALU.add)
                lk = bpool.tile([P, MB * H1], F16, tag="lk")
                nc.vector.scalar_tensor_tensor(
                    out=lk[:], in0=av[:], scalar=NEG_SLOPE, in1=av[:],
                    op0=ALU.mult, op1=ALU.max)
                efull = bpool.tile([P, MB * HC1], F16, tag="efull")
                nc.scalar.activation(
                    out=efull[:],
                    in_=lk[:].rearrange("p (m h) -> p m h", m=MB)
                          .to_broadcast([P, MB, H1, CH1]),
                    func=AF.Exp, bias=shifts_s[:, 0:1])
                wm = bpool.tile([P, MB * HC1], F16, tag="wm")
                nc.vector.tensor_tensor(
                    out=wm[:].rearrange("p (m c) -> p m c", m=MB),
                    in0=gv[:, :, 0:HC1],
                    in1=efull[:].rearrange("p (m c) -> p m c", m=MB),
                    op=ALU.mult)

                pacc = ps.tile([P, 272], F32, space="PSUM", tag="acc")
                pmsg = pacc[:, 0:HC1]
                pdent = ps.tile([P, H1], F32, space="PSUM", tag="den")
                pden = pdent[:]
                e4 = efull[:].rearrange("p (m h c) -> p m h c", m=MB, h=H1)
                for g in range(MB):
                    S = spool.tile([P, P], F16, tag="S")
                    nc.vector.tensor_scalar(
                        out=S[:], in0=iota_s[:], scalar1=drel[:, g:g + 1],
                        scalar2=None, op0=ALU.is_equal)
                    nc.tensor.matmul(out=pmsg, lhsT=S[:],
                                     rhs=wm[:, g * HC1:(g + 1) * HC1],
                                     start=(g == 0), stop=(g == MB - 1))
                    nc.tensor.matmul(out=pden, lhsT=S[:],
                                     rhs=e4[:, g, :, 0],
                                     start=(g == 0), stop=(g == MB - 1))

                den = opool.tile([P, H1], F32, tag="den")
                nc.vector.tensor_scalar_add(out=den[:], in0=pden,
                                            scalar1=EPS)
                rec = opool.tile([P, H1], F32, tag="rec")
                nc.vector.reciprocal(out=rec[:], in_=den[:])
                o1b = opool.tile([P, HC1], F16, tag="o1b")
                for h in range(H1):
                    sl = slice(h * CH1, (h + 1) * CH1)
                    nc.vector.scalar_tensor_tensor(
                        out=o1b[:, sl], in0=pacc[:, sl],
                        scalar=rec[:, h:h + 1], in1=b1b_s[:, sl],
                        op0=ALU.mult, op1=ALU.add)
                xn = opool.tile([P, HC1], F16, tag="xn")
                nc.gpsimd.tensor_scalar_min(out=xn[:], in0=o1b[:], scalar1=0.0)
                en = opool.tile([P, HC1], F16, tag="en")
                nc.scalar.activation(out=en[:], in_=xn[:], func=AF.Exp)
                helu = opool.tile([P, HC1], F16, tag="helu")
                nc.vector.scalar_tensor_tensor(
                    out=helu[:], in0=o1b[:], scalar=0.0, in1=en[:],
                    op0=ALU.max, op1=ALU.add)

                ph2 = ps.tile([P, 66], F32, space="PSUM", tag="ph2")
                for j in range(HC1 // P):
                    pT = ps.tile([P, P], F16, space="PSUM", tag="pT")
                    nc.tensor.transpose(out=pT[:],
                                        in_=helu[:, j * P:(j + 1) * P],
                                        identity=ident_s[:])
                    hT = opool.tile([P, P], F16, tag="hT")
                    nc.vector.tensor_scalar_add(out=hT[:], in0=pT[:],
                                                scalar1=0.0)
                    nc.tensor.matmul(out=ph2[:], lhsT=hT[:], rhs=w2aug_s[j][:],
                                     start=(j == 0), stop=(j == HC1 // P - 1))
                h2row = opool.tile([P, T2W], F16, tag="h2row")
                nc.vector.tensor_scalar_add(out=h2row[:, 0:66], in0=ph2[:],
                                            scalar1=0.0)
                nc.vector.memset(h2row[:, 66:T2W], 0.0)
                cc_writes.append(nc.sync.dma_start(
                    out=cc_in[r0:r0 + P, :], in_=h2row[:]))

            # ------------- phase 3: share layer-2 node table -------------
            nc.gpsimd.load_library(library_config.standard)
            cc = nc.gpsimd.collective_compute(
                "AllGather", ALU.bypass, replica_groups=groups,
                ins=[cc_in[:]], outs=[t2[:]])
            for w in cc_writes:
                _dep(cc, w, "allgather after cc writes")
            j2tile = cpool.tile([1, 1], F32, tag="j2")
            j2 = nc.gpsimd.memset(j2tile[:], 0.0)
            _dep(j2, cc, "layer2 gathers after allgather")
            nc.gpsimd.load_library(library_config.mlp)

            # ------------- phase 4: layer-2 edge aggregation -------------
            for b in range(BPC):
                r0 = b * P
                tlo = bpool.tile([P, MLO * 4], I32, tag="tlo2")
                nc.sync.dma_start(out=tlo[:], in_=eidx[b, :, cfg.o_lo:cfg.o_hi])
                thi = bpool.tile([P, MHI * 4], I32, tag="thi2")
                nc.sync.dma_start(out=thi[:], in_=eidx[b, :, cfg.o_hi:cfg.o_ad])
                tad = bpool.tile([P, MB * 4], I32, tag="tad2")
                nc.sync.dma_start(out=tad[:], in_=eidx[b, :, cfg.o_ad:cfg.o_dr])
                tdr = bpool.tile([P, MB], I32, tag="tdr2")
                nc.sync.dma_start(out=tdr[:], in_=eidx[b, :, cfg.o_dr:cfg.K])
                ilo = tlo[:].bitcast(I16)
                ihi = thi[:].bitcast(I16)
                iad = tad[:].bitcast(I16)
                drel = tdr[:].bitcast(F32)

                gath2 = bpool.tile([P, MB * T2W], F16, tag="gath2")
                _gpieces(nc, _dep, gath2, 0, T2W, t2[0:LO, :], ilo, MLO,
                         j2, "lo gather after table2")
                _gpieces(nc, _dep, gath2, MLO, T2W, t2[LO:NPAD, :], ihi, MHI,
                         j2, "hi gather after table2")
                adst2 = bpool.tile([P, MB * T2W], F16, tag="adst2")
                _gpieces(nc, _dep, adst2, 0, T2W, cc_in[:], iad, MB,
                         j2, "adst2 gather after cc writes")

                qv = gath2[:].rearrange("p (m w) -> p m w", m=MB)
                av2 = bpool.tile([P, MB], F16, tag="av2")
                nc.vector.scalar_tensor_tensor(
                    out=av2[:].rearrange("p (m o) -> p m o", m=MB),
                    in0=qv[:, :, 64:65], scalar=shifts_s[:, 1:2],
                    in1=adst2[:].rearrange("p (m w) -> p m w",
                                           m=MB)[:, :, 65:66],
                    op0=ALU.add, op1=ALU.add)
                lk2 = bpool.tile([P, MB], F16, tag="lk2")
                nc.vector.scalar_tensor_tensor(
                    out=lk2[:], in0=av2[:], scalar=NEG_SLOPE, in1=av2[:],
                    op0=ALU.mult, op1=ALU.max)
                e2full = bpool.tile([P, MB * C2], F16, tag="e2full")
                nc.scalar.activation(
                    out=e2full[:],
                    in_=lk2[:].rearrange("p (m o) -> p m o", m=MB)
                           .to_broadcast([P, MB, 1, C2]),
                    func=AF.Exp, bias=shifts_s[:, 0:1])
                wm2 = bpool.tile([P, MB * C2], F16, tag="wm2")
                nc.vector.tensor_tensor(
                    out=wm2[:].rearrange("p (m c) -> p m c", m=MB),
                    in0=qv[:, :, 0:C2],
                    in1=e2full[:].rearrange("p (m c) -> p m c", m=MB),
                    op=ALU.mult)

                pacc2 = ps.tile([P, 272], F32, space="PSUM", tag="acc")
                pmsg2 = pacc2[:, 0:C2]
                pdent2 = ps.tile([P, H1], F32, space="PSUM", tag="den")
                pden2 = pdent2[:, 0:1]
                for g in range(MB):
                    S = spool.tile([P, P], F16, tag="S")
                    nc.vector.tensor_scalar(
                        out=S[:], in0=iota_s[:], scalar1=drel[:, g:g + 1],
                        scalar2=None, op0=ALU.is_equal)
                    nc.tensor.matmul(out=pmsg2, lhsT=S[:],
                                     rhs=wm2[:, g * C2:(g + 1) * C2],
                                     start=(g == 0), stop=(g == MB - 1))
                    nc.tensor.matmul(out=pden2, lhsT=S[:],
                                     rhs=e2full[:, g * C2:g * C2 + 1],
                                     start=(g == 0), stop=(g == MB - 1))

                den2 = opool.tile([P, 1], F32, tag="den2")
                nc.vector.tensor_scalar_add(out=den2[:], in0=pden2,
                                            scalar1=EPS)
                rec2 = opool.tile([P, 1], F32, tag="rec2")
                nc.vector.reciprocal(out=rec2[:], in_=den2[:])
                o2 = opool.tile([P, C2], F32, tag="o2")
                nc.vector.scalar_tensor_tensor(
                    out=o2[:], in0=pmsg2, scalar=rec2[:, 0:1],
                    in1=b2b_s[:], op0=ALU.mult, op1=ALU.add)
                nc.sync.dma_start(out=out[r0:r0 + P, :], in_=o2[:])

    nc.compile()
    return nc


def _wrap16(idx, nid):
    """Pack an int16 index list (len nid) into a [128, nid//16] tile:
    element k at (k%16, k//16), replicated to partitions 16..127."""
    a = np.asarray(idx, np.int16).reshape(nid // 16, 16).T  # [16, nid//16]
    return np.tile(a, (8, 1))


def host_prep(cfg, edge_index):
    n = cfg.n
    src = np.asarray(edge_index[0]).astype(np.int64)
    dst = np.asarray(edge_index[1]).astype(np.int64)
    loop = np.arange(n, dtype=np.int64)
    src = np.concatenate([src, loop])
    dst = np.concatenate([dst, loop])

    order = np.argsort(dst, kind="stable")
    ss = src[order]
    ds = dst[order]
    blk = ds >> 7

    MLO, MHI, MB = cfg.mlo, cfg.mhi, cfg.mb
    NBLK, BPC = cfg.nblk, cfg.bpc
    eidx = np.zeros((NBLK, P, cfg.K), dtype=np.int32)

    starts = np.zeros(NBLK + 1, dtype=np.int64)
    np.cumsum(np.bincount(blk, minlength=NBLK), out=starts[1:])

    for B in range(NBLK):
        s_b = ss[starts[B]:starts[B + 1]]
        d_b = ds[starts[B]:starts[B + 1]]
        lo_m = s_b < LO
        s_lo, d_lo = s_b[lo_m], d_b[lo_m]
        s_hi, d_hi = s_b[~lo_m], d_b[~lo_m]
        nlo, nhi = len(s_lo), len(s_hi)
        assert -(-nlo // P) <= MLO and -(-nhi // P) <= MHI, (B, nlo, nhi)
        cbase = (B // BPC) * BPC * P

        ilo = np.zeros(MLO * P, np.int16)
        ilo[:nlo] = s_lo.astype(np.int16)
        ihi = np.zeros(MHI * P, np.int16)
        ihi[:nhi] = (s_hi - LO).astype(np.int16)
        kidx = np.concatenate([np.arange(nlo), MLO * P + np.arange(nhi)])
        d_all = np.concatenate([d_lo, d_hi])
        iad = np.zeros(MB * P, np.int64)
        iad[kidx] = d_all - cbase
        drel = np.full((P, MB), -1.0, dtype=np.float32)
        drel[kidx % P, kidx // P] = (d_all - (B << 7)).astype(np.float32)

        eidx[B, :, cfg.o_lo:cfg.o_hi] = np.ascontiguousarray(
            _wrap16(ilo, MLO * P)).view(np.int32)
        eidx[B, :, cfg.o_hi:cfg.o_ad] = np.ascontiguousarray(
            _wrap16(ihi, MHI * P)).view(np.int32)
        eidx[B, :, cfg.o_ad:cfg.o_dr] = np.ascontiguousarray(
            _wrap16(iad.astype(np.int16), MB * P)).view(np.int32)
        eidx[B, :, cfg.o_dr:cfg.K] = drel.view(np.int32)

    return [np.ascontiguousarray(eidx[c * BPC:(c + 1) * BPC])
            for c in range(cfg.ncores)]


def compute_m(n, edge_index):
    src = np.asarray(edge_index[0]).astype(np.int64)
    dst = np.asarray(edge_index[1]).astype(np.int64)
    loop = np.arange(n, dtype=np.int64)
    src = np.concatenate([src, loop])
    dst = np.concatenate([dst, loop])
    blk = dst >> 7
    nblk = -(-n // P)
    lo = src < LO
    clo = np.bincount(blk[lo], minlength=nblk)
    chi = np.bincount(blk[~lo], minlength=nblk)
    return int(-(-clo.max() // P)), int(-(-chi.max() // P))


def make_in_maps(cfg, x, W1, att_src1, att_dst1, bias1, W2, att_src2,
                 att_dst2, bias2, edge_index):
    H1, CH1, HC1, C2 = cfg.h1, cfg.ch1, cfg.hc1, cfg.c2
    x = np.asarray(x, dtype=np.float32)
    xpad = np.zeros((cfg.npad, cfg.c_in), dtype=np.float32)
    xpad[: cfg.n] = x
    xt = np.ascontiguousarray(xpad.T).astype(np.float16)

    W1 = np.asarray(W1, np.float32)
    W2 = np.asarray(W2, np.float32)
    as1 = np.asarray(att_src1, np.float32)
    ad1 = np.asarray(att_dst1, np.float32)
    as2 = np.asarray(att_src2, np.float32).reshape(-1)
    ad2 = np.asarray(att_dst2, np.float32).reshape(-1)

    A1s = np.zeros((HC1, H1), dtype=np.float32)
    A1d = np.zeros((HC1, H1), dtype=np.float32)
    hh = np.repeat(np.arange(H1), CH1)
    A1s[np.arange(HC1), hh] = as1.reshape(-1)
    A1d[np.arange(HC1), hh] = ad1.reshape(-1)
    w1aug = np.concatenate([W1, W1 @ A1s, W1 @ A1d], axis=1).astype(np.float16)
    w2aug = np.concatenate([W2, (W2 @ as2)[:, None], (W2 @ ad2)[:, None]],
                           axis=1).astype(np.float16)

    colsum = W2.sum(axis=0)
    c0 = float(colsum @ (as2 + ad2))
    shifts = np.zeros((P, 2), dtype=np.float32)
    shifts[:, 0] = -2.0   # exp bias (cancels in softmax; keeps fp16 safe)
    shifts[:, 1] = -c0    # undo eluplus fold's logit shift (pre-leaky)

    b1b = np.tile(np.asarray(bias1, np.float32).reshape(1, -1),
                  (P, 1)).astype(np.float16)
    b2b = np.tile((np.asarray(bias2, np.float32).reshape(-1) - colsum
                   ).reshape(1, -1), (P, 1)).astype(np.float32)
    iota = np.tile(np.arange(P, dtype=np.float16), (P, 1))
    ident = np.eye(P, dtype=np.float16)

    per_core = host_prep(cfg, edge_index)
    G0 = cfg.nloc // (8 * P)
    in_maps = []
    for c in range(cfg.ncores):
        m = {"xt": xt, "w1aug": w1aug, "w2aug": w2aug, "b1b": b1b,
             "b2b": b2b, "iota": iota, "ident": ident, "shifts": shifts}
        base = c * cfg.bpc * P
        cols = np.minimum(base + np.arange(G0) * 8 * P,
                          cfg.npad - 8 * P).astype(np.int64)
        xoff = (np.arange(P)[:, None] * cfg.npad +
                cols[None, :]).astype(np.int32)
        m["xoff"] = xoff
        m["eidx"] = per_core[c]
        in_maps.append(m)
    return in_maps


_prog_cache = {}
_last_results = None


def kernel(x, edge_index, edge_weight, W1, att_src1, att_dst1, bias1,
           W2, att_src2, att_dst2, bias2):
    global _last_results
    n = x.shape[0]
    # edge_weight is unused (GATConv with edge_dim=None ignores it)
    mlo, mhi = compute_m(n, edge_index)
    mlo, mhi = max(mlo, 13), max(mhi, 8)

    cfg = Cfg(n, mlo, mhi, c_in=x.shape[1], h1=8, ch1=32, c2=64)
    key = (cfg.n, cfg.c_in, cfg.mlo, cfg.mhi)
    if key not in _prog_cache:
        _prog_cache[key] = build_program(cfg)
    nc = _prog_cache[key]

    in_maps = make_in_maps(cfg, x, W1, att_src1, att_dst1, bias1, W2,
                           att_src2, att_dst2, bias2, edge_index)
    res = run_bass_kernel_spmd(nc, in_maps, list(range(cfg.ncores)))
    _last_results = res
    outs = [res.results[c]["out"] for c in range(cfg.ncores)]
    full = np.concatenate(outs, axis=0)[: cfg.n]
    return np.ascontiguousarray(full)


# revision 21
# speedup vs baseline: 1871.9027x; 1.0886x over previous
"""GAT (2-layer, PyG-style) Trainium2 Bass kernel, 8-core SPMD. v3.

Strategy (dst-sharded edge aggregation, fp16 edge path):
- Host sorts edges by dst into 128-node blocks; within a block edges are
  ordered [src<32768 | src>=32768], each group padded to a multiple of 128
  (MLO/MHI chunks of 128 edge slots, global maxima). Core c owns blocks
  [c*BPC, (c+1)*BPC) and produces output rows for its own nodes only.
- Node tables live in DRAM with rows padded to 256B multiples so that
  InstDMAGatherAnt (int16 indices, one descriptor per edge, ~1us fixed cost
  per call) batches a whole block's gathers into 2 calls (low/high rows).
- a_dst is gathered from small per-core local tables whose indices fit
  int16: t1loc (written by phase 0) for layer 1, cc_in itself for layer 2.
- Edge softmax + scatter-add are expressed via one-hot S matrices + PE
  matmuls (fp16 in, fp32 PSUM accumulate). Logits are exp-shifted by -2
  (cancels exactly in softmax; keeps fp16 exp finite).
- ELU's "-1" is dropped (eluplus = relu(x)+exp(min(x,0))) and corrected at
  the end: out -= colsum(W2) (exact since softmax weights sum to 1); the
  induced constant layer-2 logit shift c0 is subtracted pre-leaky_relu.
- One AllGather of the 128-col fp16 layer-2 table is the only collective.
"""

import numpy as np

import concourse.bacc as bacc
import concourse.bass as bass
import concourse.mybir as mybir
import concourse.tile as tile
from concourse import library_config
from concourse.bass_utils import run_bass_kernel_spmd
from bass_rust import add_dep_helper


def _dep(a, b, reason):
    ia = a.ins if hasattr(a, "ins") else a
    ib = b.ins if hasattr(b, "ins") else b
    add_dep_helper(ia, ib, reason=reason)


P = 128
NCORES = 8
EPS = 1e-16
NEG_SLOPE = 0.2
LO = 32768                  # int16 index limit -> low/high table split
F32 = mybir.dt.float32
F16 = mybir.dt.float16
I32 = mybir.dt.int32
I16 = mybir.dt.int16
AF = mybir.ActivationFunctionType
ALU = mybir.AluOpType


class Cfg:
    def __init__(self, n_nodes, mlo, mhi, c_in=128, h1=8, ch1=32, c2=64,
                 ncores=NCORES):
        self.n = n_nodes
        self.c_in = c_in
        self.h1 = h1
        self.ch1 = ch1
        self.hc1 = h1 * ch1          # 256
        self.c2 = c2
        self.ncores = ncores
        self.bpc = -(-n_nodes // (P * ncores))      # 49
        self.npad = ncores * self.bpc * P
        self.nblk = ncores * self.bpc
        self.mlo = mlo
        self.mhi = mhi
        self.mb = mlo + mhi
        self.t1w = 384               # [h 256 | asrc 8 | adst 8 | pad]
        self.t2w = 128               # [h2 64 | asrc2 1 | adst2 1 | pad]
        self.nloc = ((self.bpc + 7) // 8) * 8 * P   # t1loc rows
        # eidx blob layout per block, in i32 columns:
        self.o_lo = 0                          # srclo idx16: mlo*4 i32 cols
        self.o_hi = self.o_lo + mlo * 4        # srchi idx16: mhi*4
        self.o_ad = self.o_hi + mhi * 4        # adst idx16:  mb*4
        self.o_dr = self.o_ad + self.mb * 4    # drel f32:    mb
        self.K = self.o_dr + self.mb



_GQ = [0]


def _gpieces(nc, dep_fn, out_tile, cbase, W, in_ap, idx16, nchunks, dep, why):
    """Emit dma_gathers in <=1024-index pieces, alternating SWDGE queues."""
    for k0 in range(0, nchunks, 8):
        nk = min(8, nchunks - k0)
        q = _GQ[0] % 4
        _GQ[0] += 1
        g = nc.gpsimd.dma_gather(
            out_ap=out_tile[:, (cbase + k0) * W:(cbase + k0 + nk) * W]
                .rearrange("p (m w) -> p m w", m=nk),
            in_ap=in_ap, idxs_ap=idx16[:, k0 * 8:(k0 + nk) * 8],
            num_idxs=nk * 128, num_idxs_reg=nk * 128, elem_size=W,
            queue_num=q)
        dep_fn(g, dep, why)


def build_program(cfg):
    nc = bacc.Bacc(None, num_devices=cfg.ncores, num_swdge_queues=4)
    HC1, H1, CH1, C2 = cfg.hc1, cfg.h1, cfg.ch1, cfg.c2
    T1W, T2W, BPC = cfg.t1w, cfg.t2w, cfg.bpc
    MLO, MHI, MB = cfg.mlo, cfg.mhi, cfg.mb
    NBLK, NPAD, NLOC = cfg.nblk, cfg.npad, cfg.nloc
    G1 = 8
    assert NBLK % G1 == 0
    G0 = NLOC // (G1 * P)

    # ---- I/O ----
    xt = nc.dram_tensor("xt", [cfg.c_in, NPAD], F16, kind="ExternalInput")
    w1aug = nc.dram_tensor("w1aug", [cfg.c_in, 272], F16, kind="ExternalInput")
    w2aug = nc.dram_tensor("w2aug", [HC1, 66], F16, kind="ExternalInput")
    b1b = nc.dram_tensor("b1b", [P, HC1], F16, kind="ExternalInput")
    b2b = nc.dram_tensor("b2b", [P, C2], F32, kind="ExternalInput")
    iota = nc.dram_tensor("iota", [P, P], F16, kind="ExternalInput")
    ident = nc.dram_tensor("ident", [P, P], F16, kind="ExternalInput")
    shifts = nc.dram_tensor("shifts", [P, 2], F32, kind="ExternalInput")
    xoff = nc.dram_tensor("xoff", [P, G0], I32, kind="ExternalInput")
    eidx = nc.dram_tensor("eidx", [BPC, P, cfg.K], I32, kind="ExternalInput")
    out = nc.dram_tensor("out", [BPC * P, C2], F32, kind="ExternalOutput")

    # ---- internal DRAM ----
    t1 = nc.dram_tensor("t1", [NPAD, T1W], F16)
    t1loc = nc.dram_tensor("t1loc", [NLOC, T2W], F16)
    cc_in = nc.dram_tensor("cc_in", [BPC * P, T2W], F16)
    t2 = nc.dram_tensor("t2", [NPAD, T2W], F16, addr_space="Shared")

    groups = [list(range(cfg.ncores))]

    with tile.TileContext(nc) as tc:
        with (
            tc.tile_pool(name="const", bufs=1) as cpool,
            tc.tile_pool(name="p1", bufs=2) as p1pool,
            tc.tile_pool(name="blk", bufs=2) as bpool,
            tc.tile_pool(name="s", bufs=4) as spool,
            tc.tile_pool(name="fin", bufs=2) as opool,
            tc.tile_pool(name="ps", bufs=2, space="PSUM") as ps,
        ):
            # ---------------- constants ----------------
            iota_s = cpool.tile([P, P], F16)
            nc.sync.dma_start(out=iota_s[:], in_=iota[:])
            ident_s = cpool.tile([P, P], F16)
            nc.sync.dma_start(out=ident_s[:], in_=ident[:])
            w1aug_s = cpool.tile([P, 272], F16)
            nc.sync.dma_start(out=w1aug_s[:], in_=w1aug[:])
            w2aug_s = []
            for j in range(HC1 // P):
                wg = cpool.tile([P, 66], F16, tag=f"w2aug{j}")
                nc.sync.dma_start(out=wg[:], in_=w2aug[j * P:(j + 1) * P, :])
                w2aug_s.append(wg)
            b1b_s = cpool.tile([P, HC1], F16)
            nc.sync.dma_start(out=b1b_s[:], in_=b1b[:])
            b2b_s = cpool.tile([P, C2], F32)
            nc.sync.dma_start(out=b2b_s[:], in_=b2b[:])
            shifts_s = cpool.tile([P, 2], F32)
            nc.sync.dma_start(out=shifts_s[:], in_=shifts[:])
            xoff_s = cpool.tile([P, G0], I32)
            nc.sync.dma_start(out=xoff_s[:], in_=xoff[:])

            # ------------- phase 0: local a_dst table (own blocks) -------
            loc_writes = []
            for g in range(G0):
                xg0 = p1pool.tile([P, G1 * P], F16, tag="xg0")
                nc.gpsimd.indirect_dma_start(
                    out=xg0[:], out_offset=None, in_=xt[:],
                    in_offset=bass.IndirectOffsetOnAxis(
                        ap=xoff_s[:, g:g + 1], axis=1))
                rows0 = p1pool.tile([P, G1 * H1], F16, tag="rows0")
                for j in range(G1):
                    pl = ps.tile([P, H1], F32, space="PSUM", tag="den")
                    nc.tensor.matmul(out=pl[:], lhsT=xg0[:, j * P:(j + 1) * P],
                                     rhs=w1aug_s[:, 264:272],
                                     start=True, stop=True)
                    nc.vector.tensor_scalar_add(
                        out=rows0[:, j * H1:(j + 1) * H1], in0=pl[:],
                        scalar1=0.0)
                loc_writes.append(nc.sync.dma_start(
                    out=t1loc[g * G1 * P:(g + 1) * G1 * P, 0:H1].rearrange(
                        "(j p) c -> p j c", j=G1),
                    in_=rows0[:].rearrange("p (j c) -> p j c", j=G1)))
            jloc_t = cpool.tile([1, 1], F32, tag="jloc")
            jloc = nc.gpsimd.memset(jloc_t[:], 0.0)
            for w in loc_writes:
                _dep(jloc, w, "adst gathers wait for local table")

            # ------------- phase 1: full node table (replicated) ---------
            t1_writes = []
            for grp in range(NBLK // G1):
                B0 = grp * G1
                xg = p1pool.tile([P, G1 * P], F16, tag="xg")
                nc.sync.dma_start(out=xg[:], in_=xt[:, B0 * P:(B0 + G1) * P])
                rows = p1pool.tile([P, G1 * 272], F16, tag="rows")
                for j in range(G1):
                    ph1 = ps.tile([P, 272], F32, space="PSUM", tag="acc")
                    nc.tensor.matmul(out=ph1[:], lhsT=xg[:, j * P:(j + 1) * P],
                                     rhs=w1aug_s[:], start=True, stop=True)
                    dst = rows[:, j * 272:(j + 1) * 272]
                    if j % 2 == 0:
                        nc.scalar.copy(out=dst, in_=ph1[:])
                    else:
                        nc.vector.tensor_scalar_add(out=dst, in0=ph1[:],
                                                    scalar1=0.0)
                t1_writes.append(nc.sync.dma_start(
                    out=t1[B0 * P:(B0 + G1) * P, 0:272].rearrange(
                        "(j p) c -> p j c", j=G1),
                    in_=rows[:].rearrange("p (j c) -> p j c", j=G1)))

            j1tile = cpool.tile([1, 1], F32, tag="j1")
            j1 = nc.gpsimd.memset(j1tile[:], 0.0)
            for w in t1_writes:
                _dep(j1, w, "layer1 gathers wait for full node table")

            # gpsimd ext-isa library containing InstDMAGatherAnt
            nc.gpsimd.load_library(library_config.mlp)

            # ------------- phase 2: layer-1 edge aggregation -------------
            cc_writes = []
            for b in range(BPC):
                r0 = b * P
                tlo = bpool.tile([P, MLO * 4], I32, tag="tlo")
                nc.sync.dma_start(out=tlo[:], in_=eidx[b, :, cfg.o_lo:cfg.o_hi])
                thi = bpool.tile([P, MHI * 4], I32, tag="thi")
                nc.sync.dma_start(out=thi[:], in_=eidx[b, :, cfg.o_hi:cfg.o_ad])
                tad = bpool.tile([P, MB * 4], I32, tag="tad")
                nc.sync.dma_start(out=tad[:], in_=eidx[b, :, cfg.o_ad:cfg.o_dr])
                tdr = bpool.tile([P, MB], I32, tag="tdr")
                nc.sync.dma_start(out=tdr[:], in_=eidx[b, :, cfg.o_dr:cfg.K])
                ilo = tlo[:].bitcast(I16)
                ihi = thi[:].bitcast(I16)
                iad = tad[:].bitcast(I16)
                drel = tdr[:].bitcast(F32)

                gath = bpool.tile([P, MB * T1W], F16, tag="gath")
                _gpieces(nc, _dep, gath, 0, T1W, t1[0:LO, :], ilo, MLO,
                         j1, "lo gather after table1")
                _gpieces(nc, _dep, gath, MLO, T1W, t1[LO:NPAD, :], ihi, MHI,
                         j1, "hi gather after table1")
                adstg = bpool.tile([P, MB * T2W], F16, tag="adstg")
                _gpieces(nc, _dep, adstg, 0, T2W, t1loc[:], iad, MB,
                         jloc, "adst gather after local table")

                gv = gath[:].rearrange("p (m w) -> p m w", m=MB)
                av = bpool.tile([P, MB * H1], F16, tag="av")
                nc.vector.tensor_tensor(
                    out=av[:].rearrange("p (m h) -> p m h", m=MB),
                    in0=gv[:, :, 256:264],
                    in1=adstg[:].rearrange("p (m w) -> p m w",
                                           m=MB)[:, :, 0:H1],
                    op=ALU.add)
                lk = bpool.tile([P, MB * H1], F16, tag="lk")
                nc.vector.scalar_tensor_tensor(
                    out=lk[:], in0=av[:], scalar=NEG_SLOPE, in1=av[:],
                    op0=ALU.mult, op1=ALU.max)
                efull = bpool.tile([P, MB * HC1], F16, tag="efull")
                nc.scalar.activation(
                    out=efull[:],
                    in_=lk[:].rearrange("p (m h) -> p m h", m=MB)
                          .to_broadcast([P, MB, H1, CH1]),
                    func=AF.Exp, bias=shifts_s[:, 0:1])
                wm = bpool.tile([P, MB * HC1], F16, tag="wm")
                nc.vector.tensor_tensor(
                    out=wm[:].rearrange("p (m c) -> p m c", m=MB),
                    in0=gv[:, :, 0:HC1],
                    in1=efull[:].rearrange("p (m c) -> p m c", m=MB),
                    op=ALU.mult)

                pacc = ps.tile([P, 272], F32, space="PSUM", tag="acc")
                pmsg = pacc[:, 0:HC1]
                pdent = ps.tile([P, H1], F32, space="PSUM", tag="den")
                pden = pdent[:]
                e4 = efull[:].rearrange("p (m h c) -> p m h c", m=MB, h=H1)
                for g in range(MB):
                    S = spool.tile([P, P], F16, tag="S")
                    nc.vector.tensor_scalar(
                        out=S[:], in0=iota_s[:], scalar1=drel[:, g:g + 1],
                        scalar2=None, op0=ALU.is_equal)
                    nc.tensor.matmul(out=pmsg, lhsT=S[:],
                                     rhs=wm[:, g * HC1:(g + 1) * HC1],
                                     start=(g == 0), stop=(g == MB - 1))
                    nc.tensor.matmul(out=pden, lhsT=S[:],
                                     rhs=e4[:, g, :, 0],
                                     start=(g == 0), stop=(g == MB - 1))

                den = opool.tile([P, H1], F32, tag="den")
                nc.vector.tensor_scalar_add(out=den[:], in0=pden,
                                            scalar1=EPS)
                rec = opool.tile([P, H1], F32, tag="rec")
                nc.vector.reciprocal(out=rec[:], in_=den[:])
                o1b = opool.tile([P, HC1], F16, tag="o1b")
                for h in range(H1):
                    sl = slice(h * CH1, (h + 1) * CH1)
                    nc.vector.scalar_tensor_tensor(
                        out=o1b[:, sl], in0=pacc[:, sl],
                        scalar=rec[:, h:h + 1], in1=b1b_s[:, sl],
                        op0=ALU.mult, op1=ALU.add)
                xn = opool.tile([P, HC1], F16, tag="xn")
                nc.gpsimd.tensor_scalar_min(out=xn[:], in0=o1b[:], scalar1=0.0)
                en = opool.tile([P, HC1], F16, tag="en")
                nc.scalar.activation(out=en[:], in_=xn[:], func=AF.Exp)
                helu = opool.tile([P, HC1], F16, tag="helu")
                nc.vector.scalar_tensor_tensor(
                    out=helu[:], in0=o1b[:], scalar=0.0, in1=en[:],
                    op0=ALU.max, op1=ALU.add)

                ph2 = ps.tile([P, 66], F32, space="PSUM", tag="ph2")
                for j in range(HC1 // P):
                    pT = ps.tile([P, P], F16, space="PSUM", tag="pT")
                    nc.tensor.transpose(out=pT[:],
                                        in_=helu[:, j * P:(j + 1) * P],
                                        identity=ident_s[:])
                    hT = opool.tile([P, P], F16, tag="hT")
                    nc.vector.tensor_scalar_add(out=hT[:], in0=pT[:],
                                                scalar1=0.0)
                    nc.tensor.matmul(out=ph2[:], lhsT=hT[:], rhs=w2aug_s[j][:],
                                     start=(j == 0), stop=(j == HC1 // P - 1))
                h2row = opool.tile([P, T2W], F16, tag="h2row")
                nc.vector.tensor_scalar_add(out=h2row[:, 0:66], in0=ph2[:],
                                            scalar1=0.0)
                nc.vector.memset(h2row[:, 66:T2W], 0.0)
                cc_writes.append(nc.sync.dma_start(
                    out=cc_in[r0:r0 + P, :], in_=h2row[:]))

            # ------------- phase 3: share layer-2 node table -------------
            nc.gpsimd.load_library(library_config.standard)
            cc = nc.gpsimd.collective_compute(
                "AllGather", ALU.bypass, replica_groups=groups,
                ins=[cc_in[:]], outs=[t2[:]])
            for w in cc_writes:
                _dep(cc, w, "allgather after cc writes")
            j2tile = cpool.tile([1, 1], F32, tag="j2")
            j2 = nc.gpsimd.memset(j2tile[:], 0.0)
            _dep(j2, cc, "layer2 gathers after allgather")
            nc.gpsimd.load_library(library_config.mlp)

            # ------------- phase 4: layer-2 edge aggregation -------------
            for b in range(BPC):
                r0 = b * P
                tlo = bpool.tile([P, MLO * 4], I32, tag="tlo2")
                nc.sync.dma_start(out=tlo[:], in_=eidx[b, :, cfg.o_lo:cfg.o_hi])
                thi = bpool.tile([P, MHI * 4], I32, tag="thi2")
                nc.sync.dma_start(out=thi[:], in_=eidx[b, :, cfg.o_hi:cfg.o_ad])
                tad = bpool.tile([P, MB * 4], I32, tag="tad2")
                nc.sync.dma_start(out=tad[:], in_=eidx[b, :, cfg.o_ad:cfg.o_dr])
                tdr = bpool.tile([P, MB], I32, tag="tdr2")
                nc.sync.dma_start(out=tdr[:], in_=eidx[b, :, cfg.o_dr:cfg.K])
                ilo = tlo[:].bitcast(I16)
                ihi = thi[:].bitcast(I16)
                iad = tad[:].bitcast(I16)
                drel = tdr[:].bitcast(F32)

                gath2 = bpool.tile([P, MB * T2W], F16, tag="gath2")
                _gpieces(nc, _dep, gath2, 0, T2W, t2[0:LO, :], ilo, MLO,
                         j2, "lo gather after table2")
                _gpieces(nc, _dep, gath2, MLO, T2W, t2[LO:NPAD, :], ihi, MHI,
                         j2, "hi gather after table2")
                adst2 = bpool.tile([P, MB * T2W], F16, tag="adst2")
                _gpieces(nc, _dep, adst2, 0, T2W, cc_in[:], iad, MB,
                         j2, "adst2 gather after cc writes")

                qv = gath2[:].rearrange("p (m w) -> p m w", m=MB)
                av2 = bpool.tile([P, MB], F16, tag="av2")
                nc.vector.scalar_tensor_tensor(
                    out=av2[:].rearrange("p (m o) -> p m o", m=MB),
                    in0=qv[:, :, 64:65], scalar=shifts_s[:, 1:2],
                    in1=adst2[:].rearrange("p (m w) -> p m w",
                                           m=MB)[:, :, 65:66],
                    op0=ALU.add, op1=ALU.add)
                lk2 = bpool.tile([P, MB], F16, tag="lk2")
                nc.vector.scalar_tensor_tensor(
                    out=lk2[:], in0=av2[:], scalar=NEG_SLOPE, in1=av2[:],
                    op0=ALU.mult, op1=ALU.max)
                e2full = bpool.tile([P, MB * C2], F16, tag="e2full")
                nc.scalar.activation(
                    out=e2full[:],
                    in_=lk2[:].rearrange("p (m o) -> p m o", m=MB)
                           .to_broadcast([P, MB, 1, C2]),
                    func=AF.Exp, bias=shifts_s[:, 0:1])
                wm2 = bpool.tile([P, MB * C2], F16, tag="wm2")
                nc.vector.tensor_tensor(
                    out=wm2[:].rearrange("p (m c) -> p m c", m=MB),
                    in0=qv[:, :, 0:C2],
                    in1=e2full[:].rearrange("p (m c) -> p m c", m=MB),
                    op=ALU.mult)

                pacc2 = ps.tile([P, 272], F32, space="PSUM", tag="acc")
                pmsg2 = pacc2[:, 0:C2]
                pdent2 = ps.tile([P, H1], F32, space="PSUM", tag="den")
                pden2 = pdent2[:, 0:1]
                for g in range(MB):
                    S = spool.tile([P, P], F16, tag="S")
                    nc.vector.tensor_scalar(
                        out=S[:], in0=iota_s[:], scalar1=drel[:, g:g + 1],
                        scalar2=None, op0=ALU.is_equal)
                    nc.tensor.matmul(out=pmsg2, lhsT=S[:],
                                     rhs=wm2[:, g * C2:(g + 1) * C2],
                                     start=(g == 0), stop=(g == MB - 1))
                    nc.tensor.matmul(out=pden2, lhsT=S[:],
                                     rhs=e2full[:, g * C2:g * C2 + 1],
                                     start=(g == 0), stop=(g == MB - 1))

                den2 = opool.tile([P, 1], F32, tag="den2")
                nc.vector.tensor_scalar_add(out=den2[:], in0=pden2,
                                            scalar1=EPS)
                rec2 = opool.tile([P, 1], F32, tag="rec2")
                nc.vector.reciprocal(out=rec2[:], in_=den2[:])
                o2 = opool.tile([P, C2], F32, tag="o2")
                nc.vector.scalar_tensor_tensor(
                    out=o2[:], in0=pmsg2, scalar=rec2[:, 0:1],
                    in1=b2b_s[:], op0=ALU.mult, op1=ALU.add)
                nc.sync.dma_start(out=out[r0:r0 + P, :], in_=o2[:])

    nc.compile()
    return nc


def _wrap16(idx, nid):
    """Pack an int16 index list (len nid) into a [128, nid//16] tile:
    element k at (k%16, k//16), replicated to partitions 16..127."""
    a = np.asarray(idx, np.int16).reshape(nid // 16, 16).T  # [16, nid//16]
    return np.tile(a, (8, 1))


def host_prep(cfg, edge_index):
    n = cfg.n
    src = np.asarray(edge_index[0]).astype(np.int64)
    dst = np.asarray(edge_index[1]).astype(np.int64)
    loop = np.arange(n, dtype=np.int64)
    src = np.concatenate([src, loop])
    dst = np.concatenate([dst, loop])

    order = np.argsort(dst, kind="stable")
    ss = src[order]
    ds = dst[order]
    blk = ds >> 7

    MLO, MHI, MB = cfg.mlo, cfg.mhi, cfg.mb
    NBLK, BPC = cfg.nblk, cfg.bpc
    eidx = np.zeros((NBLK, P, cfg.K), dtype=np.int32)

    starts = np.zeros(NBLK + 1, dtype=np.int64)
    np.cumsum(np.bincount(blk, minlength=NBLK), out=starts[1:])

    for B in range(NBLK):
        s_b = ss[starts[B]:starts[B + 1]]
        d_b = ds[starts[B]:starts[B + 1]]
        lo_m = s_b < LO
        s_lo, d_lo = s_b[lo_m], d_b[lo_m]
        s_hi, d_hi = s_b[~lo_m], d_b[~lo_m]
        nlo, nhi = len(s_lo), len(s_hi)
        assert -(-nlo // P) <= MLO and -(-nhi // P) <= MHI, (B, nlo, nhi)
        cbase = (B // BPC) * BPC * P

        ilo = np.zeros(MLO * P, np.int16)
        ilo[:nlo] = s_lo.astype(np.int16)
        ihi = np.zeros(MHI * P, np.int16)
        ihi[:nhi] = (s_hi - LO).astype(np.int16)
        kidx = np.concatenate([np.arange(nlo), MLO * P + np.arange(nhi)])
        d_all = np.concatenate([d_lo, d_hi])
        iad = np.zeros(MB * P, np.int64)
        iad[kidx] = d_all - cbase
        drel = np.full((P, MB), -1.0, dtype=np.float32)
        drel[kidx % P, kidx // P] = (d_all - (B << 7)).astype(np.float32)

        eidx[B, :, cfg.o_lo:cfg.o_hi] = np.ascontiguousarray(
            _wrap16(ilo, MLO * P)).view(np.int32)
        eidx[B, :, cfg.o_hi:cfg.o_ad] = np.ascontiguousarray(
            _wrap16(ihi, MHI * P)).view(np.int32)
        eidx[B, :, cfg.o_ad:cfg.o_dr] = np.ascontiguousarray(
            _wrap16(iad.astype(np.int16), MB * P)).view(np.int32)
        eidx[B, :, cfg.o_dr:cfg.K] = drel.view(np.int32)

    return [np.ascontiguousarray(eidx[c * BPC:(c + 1) * BPC])
            for c in range(cfg.ncores)]


def compute_m(n, edge_index):
    src = np.asarray(edge_index[0]).astype(np.int64)
    dst = np.asarray(edge_index[1]).astype(np.int64)
    loop = np.arange(n, dtype=np.int64)
    src = np.concatenate([src, loop])
    dst = np.concatenate([dst, loop])
    blk = dst >> 7
    nblk = -(-n // P)
    lo = src < LO
    clo = np.bincount(blk[lo], minlength=nblk)
    chi = np.bincount(blk[~lo], minlength=nblk)
    return int(-(-clo.max() // P)), int(-(-chi.max() // P))


def make_in_maps(cfg, x, W1, att_src1, att_dst1, bias1, W2, att_src2,
                 att_dst2, bias2, edge_index):
    H1, CH1, HC1, C2 = cfg.h1, cfg.ch1, cfg.hc1, cfg.c2
    x = np.asarray(x, dtype=np.float32)
    xpad = np.zeros((cfg.npad, cfg.c_in), dtype=np.float32)
    xpad[: cfg.n] = x
    xt = np.ascontiguousarray(xpad.T).astype(np.float16)

    W1 = np.asarray(W1, np.float32)
    W2 = np.asarray(W2, np.float32)
    as1 = np.asarray(att_src1, np.float32)
    ad1 = np.asarray(att_dst1, np.float32)
    as2 = np.asarray(att_src2, np.float32).reshape(-1)
    ad2 = np.asarray(att_dst2, np.float32).reshape(-1)

    A1s = np.zeros((HC1, H1), dtype=np.float32)
    A1d = np.zeros((HC1, H1), dtype=np.float32)
    hh = np.repeat(np.arange(H1), CH1)
    A1s[np.arange(HC1), hh] = as1.reshape(-1)
    A1d[np.arange(HC1), hh] = ad1.reshape(-1)
    w1aug = np.concatenate([W1, W1 @ A1s, W1 @ A1d], axis=1).astype(np.float16)
    w2aug = np.concatenate([W2, (W2 @ as2)[:, None], (W2 @ ad2)[:, None]],
                           axis=1).astype(np.float16)

    colsum = W2.sum(axis=0)
    c0 = float(colsum @ (as2 + ad2))
    shifts = np.zeros((P, 2), dtype=np.float32)
    shifts[:, 0] = -2.0   # exp bias (cancels in softmax; keeps fp16 safe)
    shifts[:, 1] = -c0    # undo eluplus fold's logit shift (pre-leaky)

    b1b = np.tile(np.asarray(bias1, np.float32).reshape(1, -1),
                  (P, 1)).astype(np.float16)
    b2b = np.tile((np.asarray(bias2, np.float32).reshape(-1) - colsum
                   ).reshape(1, -1), (P, 1)).astype(np.float32)
    iota = np.tile(np.arange(P, dtype=np.float16), (P, 1))
    ident = np.eye(P, dtype=np.float16)

    per_core = host_prep(cfg, edge_index)
    G0 = cfg.nloc // (8 * P)
    in_maps = []
    for c in range(cfg.ncores):
        m = {"xt": xt, "w1aug": w1aug, "w2aug": w2aug, "b1b": b1b,
             "b2b": b2b, "iota": iota, "ident": ident, "shifts": shifts}
        base = c * cfg.bpc * P
        cols = np.minimum(base + np.arange(G0) * 8 * P,
                          cfg.npad - 8 * P).astype(np.int64)
        xoff = (np.arange(P)[:, None] * cfg.npad +
                cols[None, :]).astype(np.int32)
        m["xoff"] = xoff
        m["eidx"] = per_core[c]
        in_maps.append(m)
    return in_maps


_prog_cache = {}
_last_results = None


def kernel(x, edge_index, edge_weight, W1, att_src1, att_dst1, bias1,
           W2, att_src2, att_dst2, bias2):
    global _last_results
    n = x.shape[0]
    # edge_weight is unused (GATConv with edge_dim=None ignores it)
    mlo, mhi = compute_m(n, edge_index)
    mlo, mhi = max(mlo, 13), max(mhi, 8)

    cfg = Cfg(n, mlo, mhi, c_in=x.shape[1], h1=8, ch1=32, c2=64)
    key = (cfg.n, cfg.c_in, cfg.mlo, cfg.mhi)
    if key not in _prog_cache:
        _prog_cache[key] = build_program(cfg)
    nc = _prog_cache[key]

    in_maps = make_in_maps(cfg, x, W1, att_src1, att_dst1, bias1, W2,
                           att_src2, att_dst2, bias2, edge_index)
    res = run_bass_kernel_spmd(nc, in_maps, list(range(cfg.ncores)))
    _last_results = res
    outs = [res.results[c]["out"] for c in range(cfg.ncores)]
    full = np.concatenate(outs, axis=0)[: cfg.n]
    return np.ascontiguousarray(full)
